# revision 14
# baseline (speedup 1.0000x reference)
"""Causal GQA multi-head attention (RMSNorm-QK + RoPE) on 8 Trainium2 cores.

Sharding: (batch, kv-group). Core c owns batch c//4 and GQA group c%4,
i.e. 4 q heads + 1 kv head for one batch of 2048 tokens. Each core emits
a partial [S, D] output (row-sharded Wo); the host sums 4 partials/batch.

v1 schedule (single interleaved PE stream, ~97% target occupancy):
  proj0, proj1, attn0+wo0, proj2, attn1+wo1, proj3, attn2+wo2, attn3+wo3
with per-block epilogue work queues (rmsnorm+rope) pumped into the gaps
of the following segments so the PE never waits on the DVE/scalar chains.

Key mechanics:
  - DMA issue parallelized across engine queues (sync: x tiles, scalar:
    wqkv tiles, gpsimd: bulk prefetch of x blocks 1-3 / wo / tables);
    each dma_start costs ~0.6us of issue time on its queue, so the old
    single-queue preamble serialized ~25us of issue.
  - replicated-rsqrt: the ones-matmul sumsq psum is already broadcast
    across all 128 partitions, so sqrt/recip run on the full [128,512]
    (both are free-size-bound; same cost as a [1,512] row) and the
    row-extract + cast + PE broadcast matmuls disappear.
  - softmax normalize: reciprocal of the (replicated) rowsum psum, then
    one DVE multiply straight out of the att psum. No PE broadcast.
  - Wo evictions run on the otherwise-idle gpsimd engine.
"""

import sys

sys.path.insert(0, "/opt/trn_rl_repo")

from collections import deque
from contextlib import ExitStack

import ml_dtypes
import numpy as np

import concourse.bass as bass
import concourse.tile as tile
from concourse import bacc, mybir
from concourse.bass_utils import run_bass_kernel_spmd
from concourse.masks import make_identity

B, S, D = 2, 2048, 2048
H, HKV, DH = 16, 4, 128
P = 128
NCORES = 8
HPC = 4  # q heads per core
EPS = 1e-6
ROPE_THETA = 10000.0
BF = mybir.dt.bfloat16
F32 = mybir.dt.float32
BFNP = ml_dtypes.bfloat16

Copy = mybir.ActivationFunctionType.Copy
Exp = mybir.ActivationFunctionType.Exp
Sqrt = mybir.ActivationFunctionType.Sqrt
MULT = mybir.AluOpType.mult
ADD = mybir.AluOpType.add

NBLK = 4  # 512-token blocks
BLK = S // NBLK


def _body(ctx: ExitStack, tc: tile.TileContext, xt, wqkv, wo, cossin, gqk, out):
    nc = tc.nc

    const = ctx.enter_context(tc.tile_pool(name="const", bufs=1))
    res = ctx.enter_context(tc.tile_pool(name="res", bufs=1))
    sq_pool = ctx.enter_context(tc.tile_pool(name="sqp", bufs=3))
    srt_pool = ctx.enter_context(tc.tile_pool(name="srt", bufs=2))
    rs_pool = ctx.enter_context(tc.tile_pool(name="rsp", bufs=3))
    rope_pool = ctx.enter_context(tc.tile_pool(name="rop", bufs=2))
    exp_pool = ctx.enter_context(tc.tile_pool(name="exq", bufs=4))
    nrm_pool = ctx.enter_context(tc.tile_pool(name="nrm", bufs=2))
    att_pool = ctx.enter_context(tc.tile_pool(name="attp", bufs=2))
    osb_pool = ctx.enter_context(tc.tile_pool(name="osb", bufs=2))
    # PSUM: 8 banks = scw(2x2) + attps(1) + sumps(1) + pp(2)
    scw = ctx.enter_context(tc.tile_pool(name="scw", bufs=2, space="PSUM"))
    attps = ctx.enter_context(tc.tile_pool(name="atps", bufs=1, space="PSUM"))
    sumps = ctx.enter_context(tc.tile_pool(name="smps", bufs=1, space="PSUM"))
    pp = ctx.enter_context(tc.tile_pool(name="pp", bufs=2, space="PSUM"))

    # ---- constants / resident weights ----
    ones_sq = const.tile([P, P], BF, name="ones", tag="ones")
    nc.vector.memset(ones_sq[:], 1.0)
    ident = const.tile([P, P], BF, name="ident", tag="ident")
    make_identity(nc, ident[:])
    cossin_t = const.tile([P, 2 * S + P], BF, name="cossin", tag="cossin")
    cos_t = cossin_t[:, 0:S]
    sins_t = cossin_t[:, S:2 * S]
    mask_t = cossin_t[:, 2 * S:2 * S + P]
    gqk_t = const.tile([P, 2], F32, name="gqk", tag="gqk")
    epsq_t = const.tile([P, 1], F32, name="epsq", tag="epsq")
    nc.vector.memset(epsq_t[:], P * EPS)
    epsk_t = const.tile([P, 1], F32, name="epsk", tag="epsk")
    nc.vector.memset(epsk_t[:], EPS)

    wqkv_sb = const.tile([P, 16 * 768], BF, name="wqkv", tag="wqkv")
    wo_sb = const.tile([P, HPC * D], BF, name="wo", tag="wo")
    xt0_sb = const.tile([P, 16 * BLK], BF, name="xt0", tag="xt0")
    xtr_sb = const.tile([P, 16 * 3 * BLK], BF, name="xtr", tag="xtr")

    # resident activations, [dh, token] layouts
    qT = [res.tile([P, S], BF, name=f"qT{h}", tag=f"qT{h}") for h in range(HPC)]
    kT = res.tile([P, S], BF, name="kT", tag="kT")
    vT = res.tile([P, S], BF, name="vT", tag="vT")
    v_kd = res.tile([P, S], BF, name="vkd", tag="vkd")  # [keys, dh] chunks

    # ---- preamble DMAs ----
    # Only 8 HWDGE semaphores exist; more in-flight DMAs than that forces
    # sem reuse whose ordering waits stall the issue queues. Keep the
    # preamble at ~11 DMAs, consumed promptly and in issue order.
    # sync: the first two k-tiles (smallest possible PE-start latency)
    nc.sync.dma_start(wqkv_sb[:, 0:2 * 768], wqkv[:, 0:2, :])
    nc.sync.dma_start(xt0_sb[:, 0:2 * BLK], xt[:, 0:2, 0:BLK])
    # scalar: the rest of wqkv/x block-0, two batches each
    nc.scalar.dma_start(wqkv_sb[:, 2 * 768:9 * 768], wqkv[:, 2:9, :])
    nc.scalar.dma_start(xt0_sb[:, 2 * BLK:9 * BLK], xt[:, 2:9, 0:BLK])
    nc.scalar.dma_start(wqkv_sb[:, 9 * 768:16 * 768], wqkv[:, 9:16, :])
    nc.scalar.dma_start(xt0_sb[:, 9 * BLK:16 * BLK], xt[:, 9:16, 0:BLK])
    # gpsimd: tables + bulk prefetch in need-order
    nc.gpsimd.dma_start(cossin_t[:], cossin[:])
    nc.gpsimd.dma_start(gqk_t[:], gqk[:])
    nc.gpsimd.dma_start(xtr_sb[:, 0:16 * BLK], xt[:, :, BLK:2 * BLK])
    nc.gpsimd.dma_start(
        xtr_sb[:, 16 * BLK:2 * 16 * BLK], xt[:, :, 2 * BLK:3 * BLK])
    nc.gpsimd.dma_start(wo_sb[:], wo[:])
    nc.gpsimd.dma_start(
        xtr_sb[:, 2 * 16 * BLK:3 * 16 * BLK], xt[:, :, 3 * BLK:4 * BLK])

    def xtile(nb, k):
        if nb == 0:
            return xt0_sb[:, k * BLK:(k + 1) * BLK]
        return xtr_sb[:, ((nb - 1) * 16 + k) * BLK:((nb - 1) * 16 + k + 1) * BLK]

    def wtile(k, m):
        return wqkv_sb[:, k * 768 + m * P:k * 768 + (m + 1) * P]

    # ---- per-block epilogue queues (rmsnorm + rope), pumped into gaps ----
    EPI = [deque() for _ in range(NBLK)]

    def pump(n=1):
        for _ in range(n):
            for nb in range(NBLK):
                if EPI[nb]:
                    nxt = EPI[nb].popleft()()
                    if nxt is not None:
                        EPI[nb].append(nxt)
                    break
            else:
                return

    def drain(nb):
        while EPI[nb]:
            nxt = EPI[nb].popleft()()
            if nxt is not None:
                EPI[nb].append(nxt)

    def rope_tile(dst, cols, rsf):
        """dst = (dst*cos + rot(dst)*sin) * rsf, in place; dst is the
        [P, BLK] column view; sins has the rotation sign baked into its
        first 64 rows."""
        t1 = rope_pool.tile([P, BLK], BF, name="t1", tag="t1")
        t2 = rope_pool.tile([P, BLK], BF, name="t2", tag="t2")
        nc.vector.tensor_copy(t2[0:64, :], dst[64:128, :])
        nc.vector.tensor_copy(t2[64:128, :], dst[0:64, :])
        nc.vector.tensor_tensor(t2[:], t2[:], sins_t[:, cols], MULT)
        nc.vector.tensor_tensor(t1[:], dst[:], cos_t[:, cols], MULT)
        nc.vector.tensor_tensor(t1[:], t1[:], t2[:], ADD)
        nc.vector.tensor_tensor(dst[:], t1[:], rsf[:], MULT)

    def stageA(nb, m, ps):
        cols = slice(nb * BLK, (nb + 1) * BLK)
        if m == 5:  # v: evict now, transpose to [keys, dh] chunks later
            nc.vector.tensor_copy(vT[:, cols], ps[:])

            def stageB_v():
                pst = pp.tile([P, BLK], BF, name="pst", tag="pp")
                for i in range(4):
                    c = nb * 4 + i
                    nc.tensor.transpose(pst[:, i * P:(i + 1) * P],
                                        vT[:, c * P:(c + 1) * P], ident[:])
                nc.scalar.copy(v_kd[:, cols], pst[:])
                return None

            EPI[nb].append(stageB_v)
            return
        if m < 4:
            dst, gsl, eps_t, escale = qT[m], gqk_t[:, 0:1], epsq_t, 1.0
        else:
            dst, gsl, eps_t, escale = kT, gqk_t[:, 1:2], epsk_t, 1.0 / P
        nc.scalar.activation(dst[:, cols], ps[:], Copy, bias=0.0, scale=gsl)
        sq = sq_pool.tile([P, BLK], BF, name="sq", tag="sq")
        nc.vector.tensor_tensor(sq[:], dst[:, cols], dst[:, cols], MULT)

        def stageB():
            # sumsq replicated across partitions by the ones-matmul; sqrt
            # and reciprocal both run on the full [128,512] (free-size
            # bound, same cost as one row) so no broadcast is ever needed.
            psr = pp.tile([P, BLK], F32, name="psr", tag="pp")
            nc.tensor.matmul(psr[:], ones_sq[:], sq[:], start=True, stop=True,
                             skip_group_check=True)
            srt = srt_pool.tile([P, BLK], F32, name="srt", tag="srt")
            nc.scalar.activation(srt[:], psr[:], Sqrt, bias=eps_t[:], scale=escale)
            rsf = rs_pool.tile([P, BLK], F32, name="rsf", tag="rsf")
            nc.vector.reciprocal_approx_fast(rsf[:], srt[:])

            def stageC():
                cc = slice(nb * BLK, (nb + 1) * BLK)
                rope_tile(dst[:, cc], cc, rsf)
                return None

            return stageC

        EPI[nb].append(stageB)

    def proj_block(nb):
        if nb == 0:
            # k-outer: DMA-paced warmup; uses 6 psum banks across pools
            wide = scw.tile([P, 2 * BLK], F32, name="ps", tag="scw")
            psms = [wide[:, 0:BLK], wide[:, BLK:2 * BLK]]
            psms.append(attps.tile([P, BLK], F32, name="ps", tag="attps"))
            psms.append(sumps.tile([P, BLK], F32, name="ps", tag="sumps"))
            psms.append(pp.tile([P, BLK], F32, name="ps", tag="pp"))
            psms.append(pp.tile([P, BLK], F32, name="ps", tag="pp"))
            for k in range(16):
                for m in range(6):
                    nc.tensor.matmul(
                        psms[m], wtile(k, m), xtile(0, k),
                        start=(k == 0), stop=(k == 15), skip_group_check=True,
                    )
            for m in (5, 0, 1, 2, 3, 4):  # v first: frees its pp slot early
                stageA(0, m, psms[m])
        else:
            # k-tile first: its rope unlocks attention for all 4 heads
            for m in (4, 0, 1, 2, 3, 5):
                ps = pp.tile([P, BLK], F32, name="ps", tag="pp")
                for k in range(16):
                    nc.tensor.matmul(
                        ps[:], wtile(k, m), xtile(nb, k),
                        start=(k == 0), stop=(k == 15), skip_group_check=True,
                    )
                stageA(nb, m, ps)
                pump(2)

    # ---- attention (software-pipelined) + Wo per query block ----
    def attn_head(h, qt, atts):
        """Emit scores/exp/AV for (h, qt). Score chunks are PAIRED into a
        [128,1024] 2-bank psum tile with ONE exp per pair."""
        nkc = 4 * qt + 4
        npair = nkc // 2
        q0 = qt * BLK
        ab = {}

        def pair(p):
            ps = scw.tile([P, 2 * BLK], F32, name="psS", tag="scw")
            exs = exp_pool.tile([P, 2 * BLK], BF, name="ex", tag="ex")
            offs = []
            for j in range(2):
                kc = 2 * p + j
                off = max(0, P * kc - q0)
                offs.append(off)
                nc.tensor.matmul(
                    ps[:, j * BLK + off:(j + 1) * BLK],
                    kT[:, kc * P:(kc + 1) * P], qT[h][:, q0 + off:q0 + BLK],
                    start=True, stop=(kc < 4 * qt), skip_group_check=True,
                )
                if kc >= 4 * qt:  # diagonal block: add -30000 upper triangle
                    nc.tensor.matmul(
                        ps[:, j * BLK + off:j * BLK + off + P], ident[:], mask_t[:],
                        start=False, stop=True, skip_group_check=True,
                    )
            nc.scalar.activation(exs[:, offs[0]:], ps[:, offs[0]:], Exp)
            return p, offs, exs

        def av(p, offs, exs):
            if p == 0:
                ab["att"] = attps.tile([P, BLK], F32, name="psA", tag="attps")
                ab["sum"] = sumps.tile([P, BLK], F32, name="psB", tag="sumps")
            for j in range(2):
                kc = 2 * p + j
                off = offs[j]
                exv = exs[:, j * BLK + off:(j + 1) * BLK]
                nc.tensor.matmul(
                    ab["att"][:, off:], v_kd[:, kc * P:(kc + 1) * P], exv,
                    start=(kc == 0), stop=(kc == nkc - 1), skip_group_check=True,
                )
                nc.tensor.matmul(
                    ab["sum"][:, off:], ones_sq[:], exv,
                    start=(kc == 0), stop=(kc == nkc - 1), skip_group_check=True,
                )

        pend = []
        for p in range(npair):
            pend.append(pair(p))
            pump(1)
            if len(pend) > 1:
                av(*pend.pop(0))
        while pend:
            av(*pend.pop(0))
            pump(1)

        # normalize: rowsum psum is replicated across partitions, so one
        # reciprocal + one multiply straight out of the att psum.
        rrep = nrm_pool.tile([P, BLK], F32, name="rrep", tag="rrep")
        nc.vector.reciprocal_approx_fast(rrep[:], ab["sum"][:])
        a = att_pool.tile([P, BLK], BF, name=f"att{h}", tag=f"att{h}")
        nc.vector.tensor_tensor(a[:], ab["att"][:], rrep[:], MULT)
        atts[h] = a

    def wo_block(qt, atts):
        q0 = qt * BLK
        last = qt == NBLK - 1
        for tc4 in range(4):
            osb = osb_pool.tile([P, D], BF, name="osb", tag="osb")
            for et in range(4):
                ps = pp.tile([P, 512], F32, name="pso", tag="pp")
                for h2 in range(HPC):
                    nc.tensor.matmul(
                        ps[:], atts[h2][:, tc4 * P:(tc4 + 1) * P],
                        wo_sb[:, h2 * D + et * 512:h2 * D + (et + 1) * 512],
                        start=(h2 == 0), stop=(h2 == HPC - 1), skip_group_check=True,
                    )
                # gpsimd cannot read PSUM; alternate scalar/vector evicts
                if et % 2 == 0:
                    nc.vector.tensor_copy(osb[:, et * 512:(et + 1) * 512], ps[:])
                else:
                    nc.scalar.copy(osb[:, et * 512:(et + 1) * 512], ps[:])
                if last and tc4 == 3 and et % 2 == 1:
                    nc.sync.dma_start(
                        out[q0 + tc4 * P:q0 + (tc4 + 1) * P, (et - 1) * 512:(et + 1) * 512],
                        osb[:, (et - 1) * 512:(et + 1) * 512])
            if not (last and tc4 == 3):
                nc.sync.dma_start(out[q0 + tc4 * P:q0 + (tc4 + 1) * P, :], osb[:])
            pump(1)

    def attn_wo(qt):
        atts = [None] * HPC
        for h in range(HPC):
            attn_head(h, qt, atts)
        wo_block(qt, atts)

    # ---- interleaved schedule ----
    proj_block(0)
    proj_block(1)
    drain(0)
    attn_wo(0)
    proj_block(2)
    drain(1)
    attn_wo(1)
    proj_block(3)
    drain(2)
    attn_wo(2)
    drain(3)
    attn_wo(3)


_NC_CACHE = None


def build_nc():
    global _NC_CACHE
    if _NC_CACHE is not None:
        return _NC_CACHE
    nc = bacc.Bacc(None, target_bir_lowering=False)
    xt = nc.dram_tensor("xt", [P, 16, S], BF, kind="ExternalInput")
    wqkv = nc.dram_tensor("wqkv", [P, 16, 768], BF, kind="ExternalInput")
    wo = nc.dram_tensor("wo", [P, HPC * D], BF, kind="ExternalInput")
    cossin = nc.dram_tensor("cossin", [P, 2 * S + P], BF, kind="ExternalInput")
    gqk = nc.dram_tensor("gqk", [P, 2], F32, kind="ExternalInput")
    out = nc.dram_tensor("out", [S, D], BF, kind="ExternalOutput")
    with tile.TileContext(nc) as tc:
        with ExitStack() as ctx:
            _body(ctx, tc, xt[:], wqkv[:], wo[:], cossin[:], gqk[:], out[:])
    nc.compile()
    _NC_CACHE = nc
    return nc


def _host_tables():
    pos = np.arange(S, dtype=np.float64)
    inv_freq = 1.0 / (ROPE_THETA ** (np.arange(0, DH, 2, dtype=np.float64) / DH))
    ang = pos[:, None] * inv_freq[None, :]  # [S, 64]
    cos_s = np.concatenate([np.cos(ang), np.cos(ang)], axis=-1)  # [S, 128]
    sin_s = np.concatenate([np.sin(ang), np.sin(ang)], axis=-1)
    cos_full = np.ascontiguousarray(cos_s.T)  # [128, S]
    sins = sin_s.T.copy()
    sins[0:64] *= -1.0  # rotation sign baked in
    j = np.arange(P)[:, None]
    i = np.arange(P)[None, :]
    masktri = np.where(j <= i, 0.0, -30000.0)  # [keys, queries]
    # one [128, 2S+128] blob: [cos | sins | mask] — a single preamble DMA
    cossin = np.concatenate([cos_full, sins, masktri], axis=1).astype(BFNP)
    return cossin


def kernel(qkv, Wq, Wk, Wv, Wo, q_gamma, k_gamma):
    qkv = np.asarray(qkv, dtype=np.float32)
    Wq = np.asarray(Wq, dtype=np.float32)
    Wk = np.asarray(Wk, dtype=np.float32)
    Wv = np.asarray(Wv, dtype=np.float32)
    Wo = np.asarray(Wo, dtype=np.float32)
    q_gamma = np.asarray(q_gamma, dtype=np.float32)
    k_gamma = np.asarray(k_gamma, dtype=np.float32)

    nc = build_nc()
    cossin = _host_tables()
    gqk = np.ascontiguousarray(
        np.stack([q_gamma, k_gamma], axis=1)).astype(np.float32)  # [128, 2]
    # x^T tiles in [p, k, s] layout: element [p, k, s] = qkv[b].T[128k+p, s]
    xts = [
        np.ascontiguousarray(
            qkv[b].T.reshape(16, P, S).transpose(1, 0, 2)
        ).astype(BFNP)
        for b in range(B)
    ]

    in_maps = []
    for c in range(NCORES):
        b, g = c // 4, c % 4
        wq_c = Wq[4 * g * DH:(4 * g + 4) * DH, :]  # [512, D]
        wk_c = Wk[g * DH:(g + 1) * DH, :]  # [128, D]
        wv_c = Wv[g * DH:(g + 1) * DH, :]
        wqkv_c = np.concatenate([wq_c, wk_c, wv_c], axis=0).T  # [D, 768]
        wqkv_c = np.ascontiguousarray(
            wqkv_c.reshape(16, P, 768).transpose(1, 0, 2)).astype(BFNP)  # [128,16,768]
        wo_c = np.stack(
            [np.ascontiguousarray(Wo[:, (4 * g + h) * DH:(4 * g + h + 1) * DH].T)
             for h in range(HPC)]
        )  # [4, 128, D]
        wo_c = np.ascontiguousarray(
            wo_c.transpose(1, 0, 2).reshape(P, HPC * D)).astype(BFNP)
        in_maps.append({
            "xt": xts[b], "wqkv": wqkv_c, "wo": wo_c,
            "cossin": cossin, "gqk": gqk,
        })

    res = run_bass_kernel_spmd(nc, in_maps, core_ids=list(range(NCORES)))
    full = np.empty((B, S, D), np.float32)
    for b in range(B):
        acc = res.results[4 * b]["out"].astype(np.float32)
        for g in range(1, 4):
            acc += res.results[4 * b + g]["out"].astype(np.float32)
        full[b] = acc
    return full


# revision 18
# speedup vs baseline: 1.0178x; 1.0178x over previous
"""Causal GQA multi-head attention (RMSNorm-QK + RoPE) on 8 Trainium2 cores.

Sharding: (batch, kv-group). Core c owns batch c//4 and GQA group c%4,
i.e. 4 q heads + 1 kv head for one batch of 2048 tokens. Each core emits
a partial [S, D] output (row-sharded Wo); the host sums 4 partials/batch.

v1 schedule (single interleaved PE stream, ~97% target occupancy):
  proj0, proj1, attn0+wo0, proj2, attn1+wo1, proj3, attn2+wo2, attn3+wo3
with per-block epilogue work queues (rmsnorm+rope) pumped into the gaps
of the following segments so the PE never waits on the DVE/scalar chains.

Key mechanics:
  - DMA issue parallelized across engine queues (sync: x tiles, scalar:
    wqkv tiles, gpsimd: bulk prefetch of x blocks 1-3 / wo / tables);
    each dma_start costs ~0.6us of issue time on its queue, so the old
    single-queue preamble serialized ~25us of issue.
  - replicated-rsqrt: the ones-matmul sumsq psum is already broadcast
    across all 128 partitions, so sqrt/recip run on the full [128,512]
    (both are free-size-bound; same cost as a [1,512] row) and the
    row-extract + cast + PE broadcast matmuls disappear.
  - softmax normalize: reciprocal of the (replicated) rowsum psum, then
    one DVE multiply straight out of the att psum. No PE broadcast.
  - Wo evictions run on the otherwise-idle gpsimd engine.
"""

import sys

sys.path.insert(0, "/opt/trn_rl_repo")

from collections import deque
from contextlib import ExitStack

import ml_dtypes
import numpy as np

import concourse.bass as bass
import concourse.tile as tile
from concourse import bacc, mybir
from concourse.bass_utils import run_bass_kernel_spmd
from concourse.masks import make_identity

B, S, D = 2, 2048, 2048
H, HKV, DH = 16, 4, 128
P = 128
NCORES = 8
HPC = 4  # q heads per core
EPS = 1e-6
ROPE_THETA = 10000.0
BF = mybir.dt.bfloat16
F32 = mybir.dt.float32
BFNP = ml_dtypes.bfloat16

Copy = mybir.ActivationFunctionType.Copy
Exp = mybir.ActivationFunctionType.Exp
Ln = mybir.ActivationFunctionType.Ln
MULT = mybir.AluOpType.mult
ADD = mybir.AluOpType.add

NBLK = 4  # 512-token blocks
BLK = S // NBLK


def _body(ctx: ExitStack, tc: tile.TileContext, xt, wqkv, wo, cossin, gqk, out):
    nc = tc.nc

    const = ctx.enter_context(tc.tile_pool(name="const", bufs=1))
    res = ctx.enter_context(tc.tile_pool(name="res", bufs=1))
    sq_pool = ctx.enter_context(tc.tile_pool(name="sqp", bufs=3))
    srt_pool = ctx.enter_context(tc.tile_pool(name="srt", bufs=2))
    rs_pool = ctx.enter_context(tc.tile_pool(name="rsp", bufs=3))
    rope_pool = ctx.enter_context(tc.tile_pool(name="rop", bufs=2))
    exp_pool = ctx.enter_context(tc.tile_pool(name="exq", bufs=4))
    nrm_pool = ctx.enter_context(tc.tile_pool(name="nrm", bufs=2))
    att_pool = ctx.enter_context(tc.tile_pool(name="attp", bufs=2))
    osb_pool = ctx.enter_context(tc.tile_pool(name="osb", bufs=2))
    # PSUM: 8 banks = scw(2x2) + attps(1) + sumps(1) + pp(2)
    scw = ctx.enter_context(tc.tile_pool(name="scw", bufs=2, space="PSUM"))
    attps = ctx.enter_context(tc.tile_pool(name="atps", bufs=1, space="PSUM"))
    sumps = ctx.enter_context(tc.tile_pool(name="smps", bufs=1, space="PSUM"))
    pp = ctx.enter_context(tc.tile_pool(name="pp", bufs=2, space="PSUM"))

    # ---- constants / resident weights ----
    ones_sq = const.tile([P, P], BF, name="ones", tag="ones")
    nc.vector.memset(ones_sq[:], 1.0)
    ident = const.tile([P, P], BF, name="ident", tag="ident")
    make_identity(nc, ident[:])
    cossin_t = const.tile([P, 2 * S + P], BF, name="cossin", tag="cossin")
    cos_t = cossin_t[:, 0:S]
    sins_t = cossin_t[:, S:2 * S]
    mask_t = cossin_t[:, 2 * S:2 * S + P]
    gqk_t = const.tile([P, 2], F32, name="gqk", tag="gqk")
    epsq_t = const.tile([P, 1], F32, name="epsq", tag="epsq")
    nc.vector.memset(epsq_t[:], P * EPS)
    epsk_t = const.tile([P, 1], F32, name="epsk", tag="epsk")
    nc.vector.memset(epsk_t[:], EPS)

    wqkv_sb = const.tile([P, 16 * 768], BF, name="wqkv", tag="wqkv")
    wo_sb = const.tile([P, HPC * D], BF, name="wo", tag="wo")
    xt0_sb = const.tile([P, 16 * BLK], BF, name="xt0", tag="xt0")
    xtr_sb = const.tile([P, 16 * 3 * BLK], BF, name="xtr", tag="xtr")

    # resident activations, [dh, token] layouts
    qT = [res.tile([P, S], BF, name=f"qT{h}", tag=f"qT{h}") for h in range(HPC)]
    kT = res.tile([P, S], BF, name="kT", tag="kT")
    vT = res.tile([P, S], BF, name="vT", tag="vT")
    v_kd = res.tile([P, S], BF, name="vkd", tag="vkd")  # [keys, dh] chunks

    # ---- preamble DMAs ----
    # Two constraints: (a) only 8 HWDGE semaphores exist, so more
    # in-flight DMAs than that forces sem-reuse ordering waits that stall
    # the issue queues; (b) the DMA engines round-robin across queues, so
    # bulk prefetch on a parallel queue starves the urgent proj0 feeds.
    # Everything bulk goes on ONE queue (sync) in priority order; only the
    # small tables ride a second queue.
    nc.sync.dma_start(wqkv_sb[:, 0:2 * 768], wqkv[:, 0:2, :])
    nc.sync.dma_start(xt0_sb[:, 0:2 * BLK], xt[:, 0:2, 0:BLK])
    nc.sync.dma_start(wqkv_sb[:, 2 * 768:9 * 768], wqkv[:, 2:9, :])
    nc.sync.dma_start(xt0_sb[:, 2 * BLK:9 * BLK], xt[:, 2:9, 0:BLK])
    nc.sync.dma_start(wqkv_sb[:, 9 * 768:16 * 768], wqkv[:, 9:16, :])
    nc.sync.dma_start(xt0_sb[:, 9 * BLK:16 * BLK], xt[:, 9:16, 0:BLK])
    nc.sync.dma_start(xtr_sb[:, 0:16 * BLK], xt[:, :, BLK:2 * BLK])
    nc.sync.dma_start(
        xtr_sb[:, 16 * BLK:2 * 16 * BLK], xt[:, :, 2 * BLK:3 * BLK])
    nc.sync.dma_start(wo_sb[:], wo[:])
    nc.sync.dma_start(
        xtr_sb[:, 2 * 16 * BLK:3 * 16 * BLK], xt[:, :, 3 * BLK:4 * BLK])
    # gpsimd: small tables, needed by the first epilogues (~+25us)
    nc.gpsimd.dma_start(cossin_t[:], cossin[:])
    nc.gpsimd.dma_start(gqk_t[:], gqk[:])

    def xtile(nb, k):
        if nb == 0:
            return xt0_sb[:, k * BLK:(k + 1) * BLK]
        return xtr_sb[:, ((nb - 1) * 16 + k) * BLK:((nb - 1) * 16 + k + 1) * BLK]

    def wtile(k, m):
        return wqkv_sb[:, k * 768 + m * P:k * 768 + (m + 1) * P]

    # ---- per-block epilogue queues (rmsnorm + rope), pumped into gaps ----
    EPI = [deque() for _ in range(NBLK)]

    def pump(n=1):
        for _ in range(n):
            for nb in range(NBLK):
                if EPI[nb]:
                    nxt = EPI[nb].popleft()()
                    if nxt is not None:
                        EPI[nb].append(nxt)
                    break
            else:
                return

    def drain(nb):
        while EPI[nb]:
            nxt = EPI[nb].popleft()()
            if nxt is not None:
                EPI[nb].append(nxt)

    def rope_tile(dst, cols, rsf):
        """dst = (dst*cos + rot(dst)*sin) * rsf, in place; dst is the
        [P, BLK] column view; sins has the rotation sign baked into its
        first 64 rows."""
        t1 = rope_pool.tile([P, BLK], BF, name="t1", tag="t1")
        t2 = rope_pool.tile([P, BLK], BF, name="t2", tag="t2")
        nc.vector.tensor_copy(t2[0:64, :], dst[64:128, :])
        nc.vector.tensor_copy(t2[64:128, :], dst[0:64, :])
        nc.vector.tensor_tensor(t2[:], t2[:], sins_t[:, cols], MULT)
        nc.vector.tensor_tensor(t1[:], dst[:], cos_t[:, cols], MULT)
        nc.vector.tensor_tensor(t1[:], t1[:], t2[:], ADD)
        nc.vector.tensor_tensor(dst[:], t1[:], rsf[:], MULT)

    def stageA(nb, m, ps):
        cols = slice(nb * BLK, (nb + 1) * BLK)
        if m == 5:  # v: evict now, transpose to [keys, dh] chunks later
            nc.vector.tensor_copy(vT[:, cols], ps[:])

            def stageB_v():
                pst = pp.tile([P, BLK], BF, name="pst", tag="pp")
                for i in range(4):
                    c = nb * 4 + i
                    nc.tensor.transpose(pst[:, i * P:(i + 1) * P],
                                        vT[:, c * P:(c + 1) * P], ident[:])
                nc.scalar.copy(v_kd[:, cols], pst[:])
                return None

            EPI[nb].append(stageB_v)
            return
        if m < 4:
            dst, gsl, eps_t, escale = qT[m], gqk_t[:, 0:1], epsq_t, 1.0
        else:
            dst, gsl, eps_t, escale = kT, gqk_t[:, 1:2], epsk_t, 1.0 / P
        nc.scalar.activation(dst[:, cols], ps[:], Copy, bias=0.0, scale=gsl)
        sq = sq_pool.tile([P, BLK], BF, name="sq", tag="sq")
        nc.vector.tensor_tensor(sq[:], dst[:, cols], dst[:, cols], MULT)

        def stageB():
            # sumsq replicated across partitions by the ones-matmul.
            # rsqrt = exp(-0.5*ln(v)): ln and exp share ONE activation
            # table set (natural_log_exp_and_others) with the attention
            # exp, so the scalar engine never reloads tables (1.28us per
            # reload, dozens of sqrt<->exp switches otherwise). Both run
            # on the replicated [128,512] (free-size bound, same cost as
            # one row) so no row-extract/broadcast is ever needed.
            psr = pp.tile([P, BLK], F32, name="psr", tag="pp")
            nc.tensor.matmul(psr[:], ones_sq[:], sq[:], start=True, stop=True,
                             skip_group_check=True)
            lg = srt_pool.tile([P, BLK], F32, name="lg", tag="srt")
            nc.scalar.activation(lg[:], psr[:], Ln, bias=eps_t[:], scale=escale)
            rsf = rs_pool.tile([P, BLK], BF, name="rsf", tag="rsf")
            nc.scalar.activation(rsf[:], lg[:], Exp, bias=0.0, scale=-0.5)

            def stageC():
                cc = slice(nb * BLK, (nb + 1) * BLK)
                rope_tile(dst[:, cc], cc, rsf)
                return None

            return stageC

        EPI[nb].append(stageB)

    def proj_block(nb):
        if nb == 0:
            # k-outer: DMA-paced warmup; uses 6 psum banks across pools
            wide = scw.tile([P, 2 * BLK], F32, name="ps", tag="scw")
            psms = [wide[:, 0:BLK], wide[:, BLK:2 * BLK]]
            psms.append(attps.tile([P, BLK], F32, name="ps", tag="attps"))
            psms.append(sumps.tile([P, BLK], F32, name="ps", tag="sumps"))
            psms.append(pp.tile([P, BLK], F32, name="ps", tag="pp"))
            psms.append(pp.tile([P, BLK], F32, name="ps", tag="pp"))
            for k in range(16):
                for m in range(6):
                    nc.tensor.matmul(
                        psms[m], wtile(k, m), xtile(0, k),
                        start=(k == 0), stop=(k == 15), skip_group_check=True,
                    )
            for m in (5, 0, 1, 2, 3, 4):  # v first: frees its pp slot early
                stageA(0, m, psms[m])
        else:
            # k-tile first: its rope unlocks attention for all 4 heads
            for m in (4, 0, 1, 2, 3, 5):
                ps = pp.tile([P, BLK], F32, name="ps", tag="pp")
                for k in range(16):
                    nc.tensor.matmul(
                        ps[:], wtile(k, m), xtile(nb, k),
                        start=(k == 0), stop=(k == 15), skip_group_check=True,
                    )
                stageA(nb, m, ps)
                pump(2)

    # ---- attention (software-pipelined) + Wo per query block ----
    def attn_head(h, qt, atts):
        """Emit scores/exp/AV for (h, qt). Score chunks are PAIRED into a
        [128,1024] 2-bank psum tile with ONE exp per pair."""
        nkc = 4 * qt + 4
        npair = nkc // 2
        q0 = qt * BLK
        ab = {}

        def pair(p):
            ps = scw.tile([P, 2 * BLK], F32, name="psS", tag="scw")
            exs = exp_pool.tile([P, 2 * BLK], BF, name="ex", tag="ex")
            offs = []
            for j in range(2):
                kc = 2 * p + j
                off = max(0, P * kc - q0)
                offs.append(off)
                nc.tensor.matmul(
                    ps[:, j * BLK + off:(j + 1) * BLK],
                    kT[:, kc * P:(kc + 1) * P], qT[h][:, q0 + off:q0 + BLK],
                    start=True, stop=(kc < 4 * qt), skip_group_check=True,
                )
                if kc >= 4 * qt:  # diagonal block: add -30000 upper triangle
                    nc.tensor.matmul(
                        ps[:, j * BLK + off:j * BLK + off + P], ident[:], mask_t[:],
                        start=False, stop=True, skip_group_check=True,
                    )
            nc.scalar.activation(exs[:, offs[0]:], ps[:, offs[0]:], Exp)
            return p, offs, exs

        def av(p, offs, exs):
            if p == 0:
                ab["att"] = attps.tile([P, BLK], F32, name="psA", tag="attps")
                ab["sum"] = sumps.tile([P, BLK], F32, name="psB", tag="sumps")
            for j in range(2):
                kc = 2 * p + j
                off = offs[j]
                exv = exs[:, j * BLK + off:(j + 1) * BLK]
                nc.tensor.matmul(
                    ab["att"][:, off:], v_kd[:, kc * P:(kc + 1) * P], exv,
                    start=(kc == 0), stop=(kc == nkc - 1), skip_group_check=True,
                )
                nc.tensor.matmul(
                    ab["sum"][:, off:], ones_sq[:], exv,
                    start=(kc == 0), stop=(kc == nkc - 1), skip_group_check=True,
                )

        pend = []
        for p in range(npair):
            pend.append(pair(p))
            # no pump after the LAST pair: the epilogue's DVE chain would
            # land on the DVE queue between the tail AVs and the norm that
            # frees the att/sum psum banks, stalling the next head's PE.
            if p < npair - 1:
                pump(1)
            if len(pend) > 1:
                av(*pend.pop(0))
        while pend:
            av(*pend.pop(0))

        # normalize: rowsum psum is replicated across partitions, so one
        # reciprocal + one multiply straight out of the att psum.
        rrep = nrm_pool.tile([P, BLK], F32, name="rrep", tag="rrep")
        nc.vector.reciprocal_approx_fast(rrep[:], ab["sum"][:])
        a = att_pool.tile([P, BLK], BF, name=f"att{h}", tag=f"att{h}")
        nc.vector.tensor_tensor(a[:], ab["att"][:], rrep[:], MULT)
        atts[h] = a

    def wo_block(qt, atts):
        q0 = qt * BLK
        last = qt == NBLK - 1
        for tc4 in range(4):
            osb = osb_pool.tile([P, D], BF, name="osb", tag="osb")
            for et in range(4):
                ps = pp.tile([P, 512], F32, name="pso", tag="pp")
                for h2 in range(HPC):
                    nc.tensor.matmul(
                        ps[:], atts[h2][:, tc4 * P:(tc4 + 1) * P],
                        wo_sb[:, h2 * D + et * 512:h2 * D + (et + 1) * 512],
                        start=(h2 == 0), stop=(h2 == HPC - 1), skip_group_check=True,
                    )
                # gpsimd cannot read PSUM; alternate scalar/vector evicts
                if et % 2 == 0:
                    nc.vector.tensor_copy(osb[:, et * 512:(et + 1) * 512], ps[:])
                else:
                    nc.scalar.copy(osb[:, et * 512:(et + 1) * 512], ps[:])
                if last and tc4 == 3 and et % 2 == 1:
                    nc.sync.dma_start(
                        out[q0 + tc4 * P:q0 + (tc4 + 1) * P, (et - 1) * 512:(et + 1) * 512],
                        osb[:, (et - 1) * 512:(et + 1) * 512])
            if not (last and tc4 == 3):
                nc.sync.dma_start(out[q0 + tc4 * P:q0 + (tc4 + 1) * P, :], osb[:])
            pump(1)

    def attn_wo(qt):
        atts = [None] * HPC
        for h in range(HPC):
            attn_head(h, qt, atts)
        wo_block(qt, atts)

    # ---- interleaved schedule ----
    proj_block(0)
    proj_block(1)
    drain(0)
    attn_wo(0)
    proj_block(2)
    drain(1)
    attn_wo(1)
    proj_block(3)
    drain(2)
    attn_wo(2)
    drain(3)
    attn_wo(3)


_NC_CACHE = None


def build_nc():
    global _NC_CACHE
    if _NC_CACHE is not None:
        return _NC_CACHE
    nc = bacc.Bacc(None, target_bir_lowering=False)
    xt = nc.dram_tensor("xt", [P, 16, S], BF, kind="ExternalInput")
    wqkv = nc.dram_tensor("wqkv", [P, 16, 768], BF, kind="ExternalInput")
    wo = nc.dram_tensor("wo", [P, HPC * D], BF, kind="ExternalInput")
    cossin = nc.dram_tensor("cossin", [P, 2 * S + P], BF, kind="ExternalInput")
    gqk = nc.dram_tensor("gqk", [P, 2], F32, kind="ExternalInput")
    out = nc.dram_tensor("out", [S, D], BF, kind="ExternalOutput")
    with tile.TileContext(nc) as tc:
        with ExitStack() as ctx:
            _body(ctx, tc, xt[:], wqkv[:], wo[:], cossin[:], gqk[:], out[:])
    nc.compile()
    _NC_CACHE = nc
    return nc


def _host_tables():
    pos = np.arange(S, dtype=np.float64)
    inv_freq = 1.0 / (ROPE_THETA ** (np.arange(0, DH, 2, dtype=np.float64) / DH))
    ang = pos[:, None] * inv_freq[None, :]  # [S, 64]
    cos_s = np.concatenate([np.cos(ang), np.cos(ang)], axis=-1)  # [S, 128]
    sin_s = np.concatenate([np.sin(ang), np.sin(ang)], axis=-1)
    cos_full = np.ascontiguousarray(cos_s.T)  # [128, S]
    sins = sin_s.T.copy()
    sins[0:64] *= -1.0  # rotation sign baked in
    j = np.arange(P)[:, None]
    i = np.arange(P)[None, :]
    masktri = np.where(j <= i, 0.0, -30000.0)  # [keys, queries]
    # one [128, 2S+128] blob: [cos | sins | mask] — a single preamble DMA
    cossin = np.concatenate([cos_full, sins, masktri], axis=1).astype(BFNP)
    return cossin


def kernel(qkv, Wq, Wk, Wv, Wo, q_gamma, k_gamma):
    qkv = np.asarray(qkv, dtype=np.float32)
    Wq = np.asarray(Wq, dtype=np.float32)
    Wk = np.asarray(Wk, dtype=np.float32)
    Wv = np.asarray(Wv, dtype=np.float32)
    Wo = np.asarray(Wo, dtype=np.float32)
    q_gamma = np.asarray(q_gamma, dtype=np.float32)
    k_gamma = np.asarray(k_gamma, dtype=np.float32)

    nc = build_nc()
    cossin = _host_tables()
    gqk = np.ascontiguousarray(
        np.stack([q_gamma, k_gamma], axis=1)).astype(np.float32)  # [128, 2]
    # x^T tiles in [p, k, s] layout: element [p, k, s] = qkv[b].T[128k+p, s]
    xts = [
        np.ascontiguousarray(
            qkv[b].T.reshape(16, P, S).transpose(1, 0, 2)
        ).astype(BFNP)
        for b in range(B)
    ]

    in_maps = []
    for c in range(NCORES):
        b, g = c // 4, c % 4
        wq_c = Wq[4 * g * DH:(4 * g + 4) * DH, :]  # [512, D]
        wk_c = Wk[g * DH:(g + 1) * DH, :]  # [128, D]
        wv_c = Wv[g * DH:(g + 1) * DH, :]
        wqkv_c = np.concatenate([wq_c, wk_c, wv_c], axis=0).T  # [D, 768]
        wqkv_c = np.ascontiguousarray(
            wqkv_c.reshape(16, P, 768).transpose(1, 0, 2)).astype(BFNP)  # [128,16,768]
        wo_c = np.stack(
            [np.ascontiguousarray(Wo[:, (4 * g + h) * DH:(4 * g + h + 1) * DH].T)
             for h in range(HPC)]
        )  # [4, 128, D]
        wo_c = np.ascontiguousarray(
            wo_c.transpose(1, 0, 2).reshape(P, HPC * D)).astype(BFNP)
        in_maps.append({
            "xt": xts[b], "wqkv": wqkv_c, "wo": wo_c,
            "cossin": cossin, "gqk": gqk,
        })

    res = run_bass_kernel_spmd(nc, in_maps, core_ids=list(range(NCORES)))
    full = np.empty((B, S, D), np.float32)
    for b in range(B):
        acc = res.results[4 * b]["out"].astype(np.float32)
        for g in range(1, 4):
            acc += res.results[4 * b + g]["out"].astype(np.float32)
        full[b] = acc
    return full


# revision 19
# speedup vs baseline: 1.1312x; 1.1115x over previous
"""Causal GQA multi-head attention (RMSNorm-QK + RoPE) on 8 Trainium2 cores.

Sharding: (batch, kv-group). Core c owns batch c//4 and GQA group c%4,
i.e. 4 q heads + 1 kv head for one batch of 2048 tokens. Each core emits
a partial [S, D] output (row-sharded Wo); the host sums 4 partials/batch.

v1 schedule (single interleaved PE stream, ~97% target occupancy):
  proj0, proj1, attn0+wo0, proj2, attn1+wo1, proj3, attn2+wo2, attn3+wo3
with per-block epilogue work queues (rmsnorm+rope) pumped into the gaps
of the following segments so the PE never waits on the DVE/scalar chains.

Key mechanics:
  - DMA issue parallelized across engine queues (sync: x tiles, scalar:
    wqkv tiles, gpsimd: bulk prefetch of x blocks 1-3 / wo / tables);
    each dma_start costs ~0.6us of issue time on its queue, so the old
    single-queue preamble serialized ~25us of issue.
  - replicated-rsqrt: the ones-matmul sumsq psum is already broadcast
    across all 128 partitions, so sqrt/recip run on the full [128,512]
    (both are free-size-bound; same cost as a [1,512] row) and the
    row-extract + cast + PE broadcast matmuls disappear.
  - softmax normalize: reciprocal of the (replicated) rowsum psum, then
    one DVE multiply straight out of the att psum. No PE broadcast.
  - Wo evictions run on the otherwise-idle gpsimd engine.
"""

import sys

sys.path.insert(0, "/opt/trn_rl_repo")

from collections import deque
from contextlib import ExitStack

import ml_dtypes
import numpy as np

import concourse.bass as bass
import concourse.tile as tile
from concourse import bacc, mybir
from concourse.bass_utils import run_bass_kernel_spmd
from concourse.masks import make_identity

B, S, D = 2, 2048, 2048
H, HKV, DH = 16, 4, 128
P = 128
NCORES = 8
HPC = 4  # q heads per core
EPS = 1e-6
ROPE_THETA = 10000.0
BF = mybir.dt.bfloat16
F32 = mybir.dt.float32
BFNP = ml_dtypes.bfloat16

Copy = mybir.ActivationFunctionType.Copy
Exp = mybir.ActivationFunctionType.Exp
Ln = mybir.ActivationFunctionType.Ln
MULT = mybir.AluOpType.mult
ADD = mybir.AluOpType.add

NBLK = 4  # 512-token blocks
BLK = S // NBLK


def _body(ctx: ExitStack, tc: tile.TileContext, xt, wqkv, wo, cossin, gqk, out):
    nc = tc.nc

    const = ctx.enter_context(tc.tile_pool(name="const", bufs=1))
    res = ctx.enter_context(tc.tile_pool(name="res", bufs=1))
    sq_pool = ctx.enter_context(tc.tile_pool(name="sqp", bufs=3))
    srt_pool = ctx.enter_context(tc.tile_pool(name="srt", bufs=2))
    rs_pool = ctx.enter_context(tc.tile_pool(name="rsp", bufs=3))
    rope_pool = ctx.enter_context(tc.tile_pool(name="rop", bufs=2))
    exp_pool = ctx.enter_context(tc.tile_pool(name="exq", bufs=4))
    nrm_pool = ctx.enter_context(tc.tile_pool(name="nrm", bufs=2))
    att_pool = ctx.enter_context(tc.tile_pool(name="attp", bufs=2))
    osb_pool = ctx.enter_context(tc.tile_pool(name="osb", bufs=2))
    # PSUM: 8 banks = scw(2x2) + attps(1) + sumps(1) + pp(2)
    scw = ctx.enter_context(tc.tile_pool(name="scw", bufs=2, space="PSUM"))
    attps = ctx.enter_context(tc.tile_pool(name="atps", bufs=1, space="PSUM"))
    sumps = ctx.enter_context(tc.tile_pool(name="smps", bufs=1, space="PSUM"))
    pp = ctx.enter_context(tc.tile_pool(name="pp", bufs=2, space="PSUM"))

    # ---- constants / resident weights ----
    ones_sq = const.tile([P, P], BF, name="ones", tag="ones")
    nc.vector.memset(ones_sq[:], 1.0)
    ident = const.tile([P, P], BF, name="ident", tag="ident")
    make_identity(nc, ident[:])
    cossin_t = const.tile([P, 2 * S + P], BF, name="cossin", tag="cossin")
    cos_t = cossin_t[:, 0:S]
    sins_t = cossin_t[:, S:2 * S]
    mask_t = cossin_t[:, 2 * S:2 * S + P]
    gqk_t = const.tile([P, 2], F32, name="gqk", tag="gqk")
    epsq_t = const.tile([P, 1], F32, name="epsq", tag="epsq")
    nc.vector.memset(epsq_t[:], P * EPS)
    epsk_t = const.tile([P, 1], F32, name="epsk", tag="epsk")
    nc.vector.memset(epsk_t[:], EPS)

    wqkv_sb = const.tile([P, 16 * 768], BF, name="wqkv", tag="wqkv")
    wo_sb = const.tile([P, HPC * D], BF, name="wo", tag="wo")
    xt0_sb = const.tile([P, 16 * BLK], BF, name="xt0", tag="xt0")
    xtr_sb = const.tile([P, 16 * 3 * BLK], BF, name="xtr", tag="xtr")

    # resident activations, [dh, token] layouts
    qT = [res.tile([P, S], BF, name=f"qT{h}", tag=f"qT{h}") for h in range(HPC)]
    kT = res.tile([P, S], BF, name="kT", tag="kT")
    vT = res.tile([P, S], BF, name="vT", tag="vT")
    v_kd = res.tile([P, S], BF, name="vkd", tag="vkd")  # [keys, dh] chunks

    # ---- preamble DMAs ----
    # Two constraints: (a) only 8 HWDGE semaphores exist, so more
    # in-flight DMAs than that forces sem-reuse ordering waits that stall
    # the issue queues; (b) the DMA engines round-robin across queues, so
    # bulk prefetch on a parallel queue starves the urgent proj0 feeds.
    # Everything bulk goes on ONE queue (sync) in priority order; only the
    # small tables ride a second queue.
    nc.sync.dma_start(wqkv_sb[:, 0:2 * 768], wqkv[:, 0:2, :])
    nc.sync.dma_start(xt0_sb[:, 0:2 * BLK], xt[:, 0:2, 0:BLK])
    nc.sync.dma_start(wqkv_sb[:, 2 * 768:9 * 768], wqkv[:, 2:9, :])
    nc.sync.dma_start(xt0_sb[:, 2 * BLK:9 * BLK], xt[:, 2:9, 0:BLK])
    nc.sync.dma_start(wqkv_sb[:, 9 * 768:16 * 768], wqkv[:, 9:16, :])
    nc.sync.dma_start(xt0_sb[:, 9 * BLK:16 * BLK], xt[:, 9:16, 0:BLK])
    nc.sync.dma_start(xtr_sb[:, 0:16 * BLK], xt[:, :, BLK:2 * BLK])
    nc.sync.dma_start(
        xtr_sb[:, 16 * BLK:2 * 16 * BLK], xt[:, :, 2 * BLK:3 * BLK])
    nc.sync.dma_start(wo_sb[:], wo[:])
    nc.sync.dma_start(
        xtr_sb[:, 2 * 16 * BLK:3 * 16 * BLK], xt[:, :, 3 * BLK:4 * BLK])
    # gpsimd: small tables, needed by the first epilogues (~+25us)
    nc.gpsimd.dma_start(cossin_t[:], cossin[:])
    nc.gpsimd.dma_start(gqk_t[:], gqk[:])

    def xtile(nb, k):
        if nb == 0:
            return xt0_sb[:, k * BLK:(k + 1) * BLK]
        return xtr_sb[:, ((nb - 1) * 16 + k) * BLK:((nb - 1) * 16 + k + 1) * BLK]

    def wtile(k, m):
        return wqkv_sb[:, k * 768 + m * P:k * 768 + (m + 1) * P]

    # ---- per-block epilogue queues (rmsnorm + rope), pumped into gaps ----
    EPI = [deque() for _ in range(NBLK)]

    def pump(n=1):
        for _ in range(n):
            for nb in range(NBLK):
                if EPI[nb]:
                    nxt = EPI[nb].popleft()()
                    if nxt is not None:
                        EPI[nb].append(nxt)
                    break
            else:
                return

    def drain(nb):
        while EPI[nb]:
            nxt = EPI[nb].popleft()()
            if nxt is not None:
                EPI[nb].append(nxt)

    def rope_tile(dst, cols, rsf):
        """dst = (dst*cos + rot(dst)*sin) * rsf, in place; dst is the
        [P, BLK] column view; sins has the rotation sign baked into its
        first 64 rows."""
        t1 = rope_pool.tile([P, BLK], BF, name="t1", tag="t1")
        t2 = rope_pool.tile([P, BLK], BF, name="t2", tag="t2")
        nc.vector.tensor_copy(t2[0:64, :], dst[64:128, :])
        nc.vector.tensor_copy(t2[64:128, :], dst[0:64, :])
        nc.vector.tensor_tensor(t2[:], t2[:], sins_t[:, cols], MULT)
        nc.vector.tensor_tensor(t1[:], dst[:], cos_t[:, cols], MULT)
        nc.vector.tensor_tensor(t1[:], t1[:], t2[:], ADD)
        nc.vector.tensor_tensor(dst[:], t1[:], rsf[:], MULT)

    def stageA(nb, m, ps):
        cols = slice(nb * BLK, (nb + 1) * BLK)
        if m == 5:  # v: evict now, transpose to [keys, dh] chunks later
            nc.vector.tensor_copy(vT[:, cols], ps[:])

            def stageB_v():
                pst = pp.tile([P, BLK], BF, name="pst", tag="pp")
                for i in range(4):
                    c = nb * 4 + i
                    nc.tensor.transpose(pst[:, i * P:(i + 1) * P],
                                        vT[:, c * P:(c + 1) * P], ident[:])
                nc.scalar.copy(v_kd[:, cols], pst[:])
                return None

            EPI[nb].append(stageB_v)
            return
        if m < 4:
            dst, gsl, eps_t, escale = qT[m], gqk_t[:, 0:1], epsq_t, 1.0
        else:
            dst, gsl, eps_t, escale = kT, gqk_t[:, 1:2], epsk_t, 1.0 / P
        nc.scalar.activation(dst[:, cols], ps[:], Copy, bias=0.0, scale=gsl)
        sq = sq_pool.tile([P, BLK], BF, name="sq", tag="sq")
        nc.vector.tensor_tensor(sq[:], dst[:, cols], dst[:, cols], MULT)

        def stageB():
            # sumsq replicated across partitions by the ones-matmul.
            # rsqrt = exp(-0.5*ln(v)): ln and exp share ONE activation
            # table set (natural_log_exp_and_others) with the attention
            # exp, so the scalar engine never reloads tables (1.28us per
            # reload, dozens of sqrt<->exp switches otherwise). Both run
            # on the replicated [128,512] (free-size bound, same cost as
            # one row) so no row-extract/broadcast is ever needed.
            psr = pp.tile([P, BLK], F32, name="psr", tag="pp")
            nc.tensor.matmul(psr[:], ones_sq[:], sq[:], start=True, stop=True,
                             skip_group_check=True)
            lg = srt_pool.tile([P, BLK], F32, name="lg", tag="srt")
            nc.scalar.activation(lg[:], psr[:], Ln, bias=eps_t[:], scale=escale)
            rsf = rs_pool.tile([P, BLK], BF, name="rsf", tag="rsf")
            nc.scalar.activation(rsf[:], lg[:], Exp, bias=0.0, scale=-0.5)

            def stageC():
                cc = slice(nb * BLK, (nb + 1) * BLK)
                rope_tile(dst[:, cc], cc, rsf)
                return None

            return stageC

        EPI[nb].append(stageB)

    def proj_block(nb):
        if nb == 0:
            # k-outer: DMA-paced warmup; uses 6 psum banks across pools
            wide = scw.tile([P, 2 * BLK], F32, name="ps", tag="scw")
            psms = [wide[:, 0:BLK], wide[:, BLK:2 * BLK]]
            psms.append(attps.tile([P, BLK], F32, name="ps", tag="attps"))
            psms.append(sumps.tile([P, BLK], F32, name="ps", tag="sumps"))
            psms.append(pp.tile([P, BLK], F32, name="ps", tag="pp"))
            psms.append(pp.tile([P, BLK], F32, name="ps", tag="pp"))
            for k in range(16):
                for m in range(6):
                    nc.tensor.matmul(
                        psms[m], wtile(k, m), xtile(0, k),
                        start=(k == 0), stop=(k == 15), skip_group_check=True,
                    )
            for m in (5, 0, 1, 2, 3, 4):  # v first: frees its pp slot early
                stageA(0, m, psms[m])
        else:
            # k-tile first: its rope unlocks attention for all 4 heads
            for m in (4, 0, 1, 2, 3, 5):
                ps = pp.tile([P, BLK], F32, name="ps", tag="pp")
                for k in range(16):
                    nc.tensor.matmul(
                        ps[:], wtile(k, m), xtile(nb, k),
                        start=(k == 0), stop=(k == 15), skip_group_check=True,
                    )
                stageA(nb, m, ps)
                pump(2)

    # ---- attention (software-pipelined) + Wo per query block ----
    def attn_head(h, qt, atts):
        """Emit scores/exp/AV for (h, qt). Score chunks are PAIRED into a
        [128,1024] 2-bank psum tile with ONE exp per pair."""
        nkc = 4 * qt + 4
        npair = nkc // 2
        q0 = qt * BLK
        ab = {}

        def pair(p):
            ps = scw.tile([P, 2 * BLK], F32, name="psS", tag="scw")
            exs = exp_pool.tile([P, 2 * BLK], BF, name="ex", tag="ex")
            offs = []
            for j in range(2):
                kc = 2 * p + j
                off = max(0, P * kc - q0)
                offs.append(off)
                nc.tensor.matmul(
                    ps[:, j * BLK + off:(j + 1) * BLK],
                    kT[:, kc * P:(kc + 1) * P], qT[h][:, q0 + off:q0 + BLK],
                    start=True, stop=(kc < 4 * qt), skip_group_check=True,
                )
                if kc >= 4 * qt:  # diagonal block: add -30000 upper triangle
                    nc.tensor.matmul(
                        ps[:, j * BLK + off:j * BLK + off + P], ident[:], mask_t[:],
                        start=False, stop=True, skip_group_check=True,
                    )
            nc.scalar.activation(exs[:, offs[0]:], ps[:, offs[0]:], Exp)
            return p, offs, exs

        def av(p, offs, exs):
            if p == 0:
                ab["att"] = attps.tile([P, BLK], F32, name="psA", tag="attps")
                ab["sum"] = sumps.tile([P, BLK], F32, name="psB", tag="sumps")
            for j in range(2):
                kc = 2 * p + j
                off = offs[j]
                exv = exs[:, j * BLK + off:(j + 1) * BLK]
                nc.tensor.matmul(
                    ab["att"][:, off:], v_kd[:, kc * P:(kc + 1) * P], exv,
                    start=(kc == 0), stop=(kc == nkc - 1), skip_group_check=True,
                )
                nc.tensor.matmul(
                    ab["sum"][:, off:], ones_sq[:], exv,
                    start=(kc == 0), stop=(kc == nkc - 1), skip_group_check=True,
                )

        pend = []
        for p in range(npair):
            pend.append(pair(p))
            # no pump after the LAST pair: the epilogue's DVE chain would
            # land on the DVE queue between the tail AVs and the norm that
            # frees the att/sum psum banks, stalling the next head's PE.
            if p < npair - 1:
                pump(1)
            if len(pend) > 1:
                av(*pend.pop(0))
        while pend:
            av(*pend.pop(0))

        # normalize: rowsum psum is replicated across partitions, so one
        # reciprocal + one multiply straight out of the att psum.
        rrep = nrm_pool.tile([P, BLK], F32, name="rrep", tag="rrep")
        nc.vector.reciprocal_approx_fast(rrep[:], ab["sum"][:])
        a = att_pool.tile([P, BLK], BF, name=f"att{h}", tag=f"att{h}")
        nc.vector.tensor_tensor(a[:], ab["att"][:], rrep[:], MULT)
        atts[h] = a

    def wo_block(qt, atts):
        q0 = qt * BLK
        last = qt == NBLK - 1
        for tc4 in range(4):
            osb = osb_pool.tile([P, D], BF, name="osb", tag="osb")
            for et in range(4):
                ps = pp.tile([P, 512], F32, name="pso", tag="pp")
                for h2 in range(HPC):
                    nc.tensor.matmul(
                        ps[:], atts[h2][:, tc4 * P:(tc4 + 1) * P],
                        wo_sb[:, h2 * D + et * 512:h2 * D + (et + 1) * 512],
                        start=(h2 == 0), stop=(h2 == HPC - 1), skip_group_check=True,
                    )
                # gpsimd cannot read PSUM; alternate scalar/vector evicts
                if et % 2 == 0:
                    nc.vector.tensor_copy(osb[:, et * 512:(et + 1) * 512], ps[:])
                else:
                    nc.scalar.copy(osb[:, et * 512:(et + 1) * 512], ps[:])
                if last and tc4 == 3 and et % 2 == 1:
                    nc.sync.dma_start(
                        out[q0 + tc4 * P:q0 + (tc4 + 1) * P, (et - 1) * 512:(et + 1) * 512],
                        osb[:, (et - 1) * 512:(et + 1) * 512])
            if not (last and tc4 == 3):
                nc.sync.dma_start(out[q0 + tc4 * P:q0 + (tc4 + 1) * P, :], osb[:])
            pump(1)

    def attn_wo(qt):
        atts = [None] * HPC
        for h in range(HPC):
            attn_head(h, qt, atts)
        wo_block(qt, atts)

    # ---- interleaved schedule ----
    proj_block(0)
    proj_block(1)
    drain(0)
    attn_wo(0)
    proj_block(2)
    drain(1)
    attn_wo(1)
    proj_block(3)
    drain(2)
    attn_wo(2)
    drain(3)
    attn_wo(3)


_NC_CACHE = None


def _single_act_table(nc):
    """Make every activation resolve to the one table set that holds exp,
    ln AND copy (natural_log_exp_and_others). The stock assignment maps
    each function to the FIRST containing set (exp->0, ln->5), emitting an
    alternating 1.28us ACT_TABLE_LOAD per rsqrt<->softmax switch — dozens
    per kernel. Emptying the other sets (indices preserved, so the BIR
    set-id still matches act_info.json) collapses it to one load."""
    import types
    from concourse.hw_specs import get_activation_tables

    orig = get_activation_tables(nc.m.arch)
    keep = "natural_log_exp_and_others"
    assert keep in orig, sorted(orig)
    filtered = {n: (fns if n == keep else set()) for n, fns in orig.items()}

    def patched(self):
        has_activation = any(
            isinstance(i, mybir.InstActivation)
            for b in self.main_func.blocks
            for i in b.instructions
        )
        if not has_activation:
            return
        import bass_rust as _bass_rust
        _bass_rust.insert_act_table_loads(self, list(filtered.items()))

    nc.insert_act_table_loads = types.MethodType(patched, nc)


def build_nc():
    global _NC_CACHE
    if _NC_CACHE is not None:
        return _NC_CACHE
    nc = bacc.Bacc(None, target_bir_lowering=False)
    _single_act_table(nc)
    xt = nc.dram_tensor("xt", [P, 16, S], BF, kind="ExternalInput")
    wqkv = nc.dram_tensor("wqkv", [P, 16, 768], BF, kind="ExternalInput")
    wo = nc.dram_tensor("wo", [P, HPC * D], BF, kind="ExternalInput")
    cossin = nc.dram_tensor("cossin", [P, 2 * S + P], BF, kind="ExternalInput")
    gqk = nc.dram_tensor("gqk", [P, 2], F32, kind="ExternalInput")
    out = nc.dram_tensor("out", [S, D], BF, kind="ExternalOutput")
    with tile.TileContext(nc) as tc:
        with ExitStack() as ctx:
            _body(ctx, tc, xt[:], wqkv[:], wo[:], cossin[:], gqk[:], out[:])
    nc.compile()
    _NC_CACHE = nc
    return nc


def _host_tables():
    pos = np.arange(S, dtype=np.float64)
    inv_freq = 1.0 / (ROPE_THETA ** (np.arange(0, DH, 2, dtype=np.float64) / DH))
    ang = pos[:, None] * inv_freq[None, :]  # [S, 64]
    cos_s = np.concatenate([np.cos(ang), np.cos(ang)], axis=-1)  # [S, 128]
    sin_s = np.concatenate([np.sin(ang), np.sin(ang)], axis=-1)
    cos_full = np.ascontiguousarray(cos_s.T)  # [128, S]
    sins = sin_s.T.copy()
    sins[0:64] *= -1.0  # rotation sign baked in
    j = np.arange(P)[:, None]
    i = np.arange(P)[None, :]
    masktri = np.where(j <= i, 0.0, -30000.0)  # [keys, queries]
    # one [128, 2S+128] blob: [cos | sins | mask] — a single preamble DMA
    cossin = np.concatenate([cos_full, sins, masktri], axis=1).astype(BFNP)
    return cossin


def kernel(qkv, Wq, Wk, Wv, Wo, q_gamma, k_gamma):
    qkv = np.asarray(qkv, dtype=np.float32)
    Wq = np.asarray(Wq, dtype=np.float32)
    Wk = np.asarray(Wk, dtype=np.float32)
    Wv = np.asarray(Wv, dtype=np.float32)
    Wo = np.asarray(Wo, dtype=np.float32)
    q_gamma = np.asarray(q_gamma, dtype=np.float32)
    k_gamma = np.asarray(k_gamma, dtype=np.float32)

    nc = build_nc()
    cossin = _host_tables()
    gqk = np.ascontiguousarray(
        np.stack([q_gamma, k_gamma], axis=1)).astype(np.float32)  # [128, 2]
    # x^T tiles in [p, k, s] layout: element [p, k, s] = qkv[b].T[128k+p, s]
    xts = [
        np.ascontiguousarray(
            qkv[b].T.reshape(16, P, S).transpose(1, 0, 2)
        ).astype(BFNP)
        for b in range(B)
    ]

    in_maps = []
    for c in range(NCORES):
        b, g = c // 4, c % 4
        wq_c = Wq[4 * g * DH:(4 * g + 4) * DH, :]  # [512, D]
        wk_c = Wk[g * DH:(g + 1) * DH, :]  # [128, D]
        wv_c = Wv[g * DH:(g + 1) * DH, :]
        wqkv_c = np.concatenate([wq_c, wk_c, wv_c], axis=0).T  # [D, 768]
        wqkv_c = np.ascontiguousarray(
            wqkv_c.reshape(16, P, 768).transpose(1, 0, 2)).astype(BFNP)  # [128,16,768]
        wo_c = np.stack(
            [np.ascontiguousarray(Wo[:, (4 * g + h) * DH:(4 * g + h + 1) * DH].T)
             for h in range(HPC)]
        )  # [4, 128, D]
        wo_c = np.ascontiguousarray(
            wo_c.transpose(1, 0, 2).reshape(P, HPC * D)).astype(BFNP)
        in_maps.append({
            "xt": xts[b], "wqkv": wqkv_c, "wo": wo_c,
            "cossin": cossin, "gqk": gqk,
        })

    res = run_bass_kernel_spmd(nc, in_maps, core_ids=list(range(NCORES)))
    full = np.empty((B, S, D), np.float32)
    for b in range(B):
        acc = res.results[4 * b]["out"].astype(np.float32)
        for g in range(1, 4):
            acc += res.results[4 * b + g]["out"].astype(np.float32)
        full[b] = acc
    return full


# revision 22
# speedup vs baseline: 1.1484x; 1.0151x over previous
"""Causal GQA multi-head attention (RMSNorm-QK + RoPE) on 8 Trainium2 cores.

Sharding: (batch, kv-group). Core c owns batch c//4 and GQA group c%4,
i.e. 4 q heads + 1 kv head for one batch of 2048 tokens. Each core emits
a partial [S, D] output (row-sharded Wo); the host sums 4 partials/batch.

v1 schedule (single interleaved PE stream, ~97% target occupancy):
  proj0, proj1, attn0+wo0, proj2, attn1+wo1, proj3, attn2+wo2, attn3+wo3
with per-block epilogue work queues (rmsnorm+rope) pumped into the gaps
of the following segments so the PE never waits on the DVE/scalar chains.

Key mechanics:
  - DMA issue parallelized across engine queues (sync: x tiles, scalar:
    wqkv tiles, gpsimd: bulk prefetch of x blocks 1-3 / wo / tables);
    each dma_start costs ~0.6us of issue time on its queue, so the old
    single-queue preamble serialized ~25us of issue.
  - replicated-rsqrt: the ones-matmul sumsq psum is already broadcast
    across all 128 partitions, so sqrt/recip run on the full [128,512]
    (both are free-size-bound; same cost as a [1,512] row) and the
    row-extract + cast + PE broadcast matmuls disappear.
  - softmax normalize: reciprocal of the (replicated) rowsum psum, then
    one DVE multiply straight out of the att psum. No PE broadcast.
  - Wo evictions run on the otherwise-idle gpsimd engine.
"""

import sys

sys.path.insert(0, "/opt/trn_rl_repo")

from collections import deque
from contextlib import ExitStack

import ml_dtypes
import numpy as np

import concourse.bass as bass
import concourse.tile as tile
from concourse import bacc, mybir
from concourse.bass_utils import run_bass_kernel_spmd
from concourse.masks import make_identity

B, S, D = 2, 2048, 2048
H, HKV, DH = 16, 4, 128
P = 128
NCORES = 8
HPC = 4  # q heads per core
EPS = 1e-6
ROPE_THETA = 10000.0
BF = mybir.dt.bfloat16
F32 = mybir.dt.float32
BFNP = ml_dtypes.bfloat16

Copy = mybir.ActivationFunctionType.Copy
Exp = mybir.ActivationFunctionType.Exp
Ln = mybir.ActivationFunctionType.Ln
MULT = mybir.AluOpType.mult
ADD = mybir.AluOpType.add

NBLK = 4  # 512-token blocks
BLK = S // NBLK


def _body(ctx: ExitStack, tc: tile.TileContext, xt, wqkv, wo, cossin, gqk, out):
    nc = tc.nc

    const = ctx.enter_context(tc.tile_pool(name="const", bufs=1))
    res = ctx.enter_context(tc.tile_pool(name="res", bufs=1))
    sq_pool = ctx.enter_context(tc.tile_pool(name="sqp", bufs=3))
    srt_pool = ctx.enter_context(tc.tile_pool(name="srt", bufs=2))
    rs_pool = ctx.enter_context(tc.tile_pool(name="rsp", bufs=3))
    rope_pool = ctx.enter_context(tc.tile_pool(name="rop", bufs=2))
    exp_pool = ctx.enter_context(tc.tile_pool(name="exq", bufs=4))
    nrm_pool = ctx.enter_context(tc.tile_pool(name="nrm", bufs=2))
    att_pool = ctx.enter_context(tc.tile_pool(name="attp", bufs=2))
    osb_pool = ctx.enter_context(tc.tile_pool(name="osb", bufs=2))
    # PSUM: 8 banks = scw(2x2) + attps(1) + sumps(1) + pp(2)
    scw = ctx.enter_context(tc.tile_pool(name="scw", bufs=2, space="PSUM"))
    attps = ctx.enter_context(tc.tile_pool(name="atps", bufs=1, space="PSUM"))
    sumps = ctx.enter_context(tc.tile_pool(name="smps", bufs=1, space="PSUM"))
    pp = ctx.enter_context(tc.tile_pool(name="pp", bufs=2, space="PSUM"))

    # ---- constants / resident weights ----
    ones_sq = const.tile([P, P], BF, name="ones", tag="ones")
    nc.vector.memset(ones_sq[:], 1.0)
    ident = const.tile([P, P], BF, name="ident", tag="ident")
    make_identity(nc, ident[:])
    cossin_t = const.tile([P, 2 * S + P], BF, name="cossin", tag="cossin")
    cos_t = cossin_t[:, 0:S]
    sins_t = cossin_t[:, S:2 * S]
    mask_t = cossin_t[:, 2 * S:2 * S + P]
    gqk_t = const.tile([P, 2], F32, name="gqk", tag="gqk")
    epsq_t = const.tile([P, 1], F32, name="epsq", tag="epsq")
    nc.vector.memset(epsq_t[:], P * EPS)
    epsk_t = const.tile([P, 1], F32, name="epsk", tag="epsk")
    nc.vector.memset(epsk_t[:], EPS)

    wqkv_sb = const.tile([P, 16 * 768], BF, name="wqkv", tag="wqkv")
    wo_sb = const.tile([P, HPC * D], BF, name="wo", tag="wo")
    xt0_sb = const.tile([P, 16 * BLK], BF, name="xt0", tag="xt0")
    xtr_sb = const.tile([P, 16 * 3 * BLK], BF, name="xtr", tag="xtr")

    # resident activations, [dh, token] layouts
    qT = [res.tile([P, S], BF, name=f"qT{h}", tag=f"qT{h}") for h in range(HPC)]
    kT = res.tile([P, S], BF, name="kT", tag="kT")
    vT = res.tile([P, S], BF, name="vT", tag="vT")
    v_kd = res.tile([P, S], BF, name="vkd", tag="vkd")  # [keys, dh] chunks

    # ---- preamble DMAs ----
    # Two constraints: (a) only 8 HWDGE semaphores exist, so more
    # in-flight DMAs than that forces sem-reuse ordering waits that stall
    # the issue queues; (b) the DMA engines round-robin across queues, so
    # bulk prefetch on a parallel queue starves the urgent proj0 feeds.
    # Everything bulk goes on ONE queue (sync) in priority order; only the
    # small tables ride a second queue.
    for ka, kb in ((0, 1), (1, 2), (2, 5), (5, 9), (9, 16)):
        nc.sync.dma_start(wqkv_sb[:, ka * 768:kb * 768], wqkv[:, ka:kb, :])
        nc.sync.dma_start(xt0_sb[:, ka * BLK:kb * BLK], xt[:, ka:kb, 0:BLK])
    nc.sync.dma_start(xtr_sb[:, 0:16 * BLK], xt[:, :, BLK:2 * BLK])
    nc.sync.dma_start(
        xtr_sb[:, 16 * BLK:2 * 16 * BLK], xt[:, :, 2 * BLK:3 * BLK])
    nc.sync.dma_start(wo_sb[:], wo[:])
    nc.sync.dma_start(
        xtr_sb[:, 2 * 16 * BLK:3 * 16 * BLK], xt[:, :, 3 * BLK:4 * BLK])
    # gpsimd: small tables, needed by the first epilogues (~+25us)
    nc.gpsimd.dma_start(cossin_t[:], cossin[:])
    nc.gpsimd.dma_start(gqk_t[:], gqk[:])

    def xtile(nb, k):
        if nb == 0:
            return xt0_sb[:, k * BLK:(k + 1) * BLK]
        return xtr_sb[:, ((nb - 1) * 16 + k) * BLK:((nb - 1) * 16 + k + 1) * BLK]

    def wtile(k, m):
        return wqkv_sb[:, k * 768 + m * P:k * 768 + (m + 1) * P]

    # ---- per-block epilogue queues (rmsnorm + rope), pumped into gaps ----
    EPI = [deque() for _ in range(NBLK)]

    def pump(n=1):
        for _ in range(n):
            for nb in range(NBLK):
                if EPI[nb]:
                    nxt = EPI[nb].popleft()()
                    if nxt is not None:
                        EPI[nb].append(nxt)
                    break
            else:
                return

    def drain(nb):
        while EPI[nb]:
            nxt = EPI[nb].popleft()()
            if nxt is not None:
                EPI[nb].append(nxt)

    def rope_tile(dst, cols, rsf):
        """dst = (dst*cos + rot(dst)*sin) * rsf, in place; dst is the
        [P, BLK] column view; sins has the rotation sign baked into its
        first 64 rows."""
        t1 = rope_pool.tile([P, BLK], BF, name="t1", tag="t1")
        t2 = rope_pool.tile([P, BLK], BF, name="t2", tag="t2")
        nc.vector.tensor_copy(t2[0:64, :], dst[64:128, :])
        nc.vector.tensor_copy(t2[64:128, :], dst[0:64, :])
        nc.vector.tensor_tensor(t2[:], t2[:], sins_t[:, cols], MULT)
        nc.vector.tensor_tensor(t1[:], dst[:], cos_t[:, cols], MULT)
        nc.vector.tensor_tensor(t1[:], t1[:], t2[:], ADD)
        nc.vector.tensor_tensor(dst[:], t1[:], rsf[:], MULT)

    def stageA(nb, m, ps):
        cols = slice(nb * BLK, (nb + 1) * BLK)
        if m == 5:  # v: evict now, transpose to [keys, dh] chunks later
            nc.vector.tensor_copy(vT[:, cols], ps[:])

            def stageB_v():
                pst = pp.tile([P, BLK], BF, name="pst", tag="pp")
                for i in range(4):
                    c = nb * 4 + i
                    nc.tensor.transpose(pst[:, i * P:(i + 1) * P],
                                        vT[:, c * P:(c + 1) * P], ident[:])
                nc.scalar.copy(v_kd[:, cols], pst[:])
                return None

            EPI[nb].append(stageB_v)
            return
        if m < 4:
            dst, gsl, eps_t, escale = qT[m], gqk_t[:, 0:1], epsq_t, 1.0
        else:
            dst, gsl, eps_t, escale = kT, gqk_t[:, 1:2], epsk_t, 1.0 / P
        nc.scalar.activation(dst[:, cols], ps[:], Copy, bias=0.0, scale=gsl)
        sq = sq_pool.tile([P, BLK], BF, name="sq", tag="sq")
        nc.vector.tensor_tensor(sq[:], dst[:, cols], dst[:, cols], MULT)

        def stageB():
            # sumsq replicated across partitions by the ones-matmul.
            # rsqrt = exp(-0.5*ln(v)): ln and exp share ONE activation
            # table set (natural_log_exp_and_others) with the attention
            # exp, so the scalar engine never reloads tables (1.28us per
            # reload, dozens of sqrt<->exp switches otherwise). Both run
            # on the replicated [128,512] (free-size bound, same cost as
            # one row) so no row-extract/broadcast is ever needed.
            psr = pp.tile([P, BLK], F32, name="psr", tag="pp")
            nc.tensor.matmul(psr[:], ones_sq[:], sq[:], start=True, stop=True,
                             skip_group_check=True)
            lg = srt_pool.tile([P, BLK], F32, name="lg", tag="srt")
            nc.scalar.activation(lg[:], psr[:], Ln, bias=eps_t[:], scale=escale)
            rsf = rs_pool.tile([P, BLK], BF, name="rsf", tag="rsf")
            nc.scalar.activation(rsf[:], lg[:], Exp, bias=0.0, scale=-0.5)

            def stageC():
                cc = slice(nb * BLK, (nb + 1) * BLK)
                rope_tile(dst[:, cc], cc, rsf)
                return None

            return stageC

        EPI[nb].append(stageB)

    def proj_block(nb):
        if nb == 0:
            # k-outer: DMA-paced warmup; uses 6 psum banks across pools
            wide = scw.tile([P, 2 * BLK], F32, name="ps", tag="scw")
            psms = [wide[:, 0:BLK], wide[:, BLK:2 * BLK]]
            psms.append(attps.tile([P, BLK], F32, name="ps", tag="attps"))
            psms.append(sumps.tile([P, BLK], F32, name="ps", tag="sumps"))
            psms.append(pp.tile([P, BLK], F32, name="ps", tag="pp"))
            psms.append(pp.tile([P, BLK], F32, name="ps", tag="pp"))
            for k in range(16):
                for m in range(6):
                    nc.tensor.matmul(
                        psms[m], wtile(k, m), xtile(0, k),
                        start=(k == 0), stop=(k == 15), skip_group_check=True,
                    )
            for m in (5, 0, 1, 2, 3, 4):  # v first: frees its pp slot early
                stageA(0, m, psms[m])
        else:
            # k-tile first: its rope unlocks attention for all 4 heads
            for m in (4, 0, 1, 2, 3, 5):
                ps = pp.tile([P, BLK], F32, name="ps", tag="pp")
                for k in range(16):
                    nc.tensor.matmul(
                        ps[:], wtile(k, m), xtile(nb, k),
                        start=(k == 0), stop=(k == 15), skip_group_check=True,
                    )
                stageA(nb, m, ps)
                pump(2)

    # ---- attention (software-pipelined) + Wo per query block ----
    def attn_head(h, qt, atts):
        """Emit scores/exp/AV for (h, qt). Score chunks are PAIRED into a
        [128,1024] 2-bank psum tile with ONE exp per pair."""
        nkc = 4 * qt + 4
        npair = nkc // 2
        q0 = qt * BLK
        ab = {}

        def pair(p):
            ps = scw.tile([P, 2 * BLK], F32, name="psS", tag="scw")
            exs = exp_pool.tile([P, 2 * BLK], BF, name="ex", tag="ex")
            offs = []
            for j in range(2):
                kc = 2 * p + j
                off = max(0, P * kc - q0)
                offs.append(off)
                nc.tensor.matmul(
                    ps[:, j * BLK + off:(j + 1) * BLK],
                    kT[:, kc * P:(kc + 1) * P], qT[h][:, q0 + off:q0 + BLK],
                    start=True, stop=(kc < 4 * qt), skip_group_check=True,
                )
                if kc >= 4 * qt:  # diagonal block: add -30000 upper triangle
                    nc.tensor.matmul(
                        ps[:, j * BLK + off:j * BLK + off + P], ident[:], mask_t[:],
                        start=False, stop=True, skip_group_check=True,
                    )
            nc.scalar.activation(exs[:, offs[0]:], ps[:, offs[0]:], Exp)
            return p, offs, exs

        def av(p, offs, exs):
            if p == 0:
                ab["att"] = attps.tile([P, BLK], F32, name="psA", tag="attps")
                ab["sum"] = sumps.tile([P, BLK], F32, name="psB", tag="sumps")
            for j in range(2):
                kc = 2 * p + j
                off = offs[j]
                exv = exs[:, j * BLK + off:(j + 1) * BLK]
                nc.tensor.matmul(
                    ab["att"][:, off:], v_kd[:, kc * P:(kc + 1) * P], exv,
                    start=(kc == 0), stop=(kc == nkc - 1), skip_group_check=True,
                )
                nc.tensor.matmul(
                    ab["sum"][:, off:], ones_sq[:], exv,
                    start=(kc == 0), stop=(kc == nkc - 1), skip_group_check=True,
                )

        # NO pumping inside attention: the scalar engine is ~95% busy with
        # the exp stream here (1.1us exp vs 1.28us PE per pair) and the
        # DVE must run the norm promptly to free the att/sum psum banks —
        # epilogue work injected into either queue stalls the PE.
        pend = []
        for p in range(npair):
            pend.append(pair(p))
            if len(pend) > 1:
                av(*pend.pop(0))
        while pend:
            av(*pend.pop(0))

        # normalize: rowsum psum is replicated across partitions, so one
        # reciprocal + one multiply straight out of the att psum.
        rrep = nrm_pool.tile([P, BLK], F32, name="rrep", tag="rrep")
        nc.vector.reciprocal_approx_fast(rrep[:], ab["sum"][:])
        a = att_pool.tile([P, BLK], BF, name=f"att{h}", tag=f"att{h}")
        nc.vector.tensor_tensor(a[:], ab["att"][:], rrep[:], MULT)
        atts[h] = a

    def wo_block(qt, atts):
        q0 = qt * BLK
        last = qt == NBLK - 1
        for tc4 in range(4):
            osb = osb_pool.tile([P, D], BF, name="osb", tag="osb")
            for et in range(4):
                ps = pp.tile([P, 512], F32, name="pso", tag="pp")
                for h2 in range(HPC):
                    nc.tensor.matmul(
                        ps[:], atts[h2][:, tc4 * P:(tc4 + 1) * P],
                        wo_sb[:, h2 * D + et * 512:h2 * D + (et + 1) * 512],
                        start=(h2 == 0), stop=(h2 == HPC - 1), skip_group_check=True,
                    )
                # evicts on vector only: scalar must stay clear for the
                # next attention block's exp stream (gpsimd can't read
                # PSUM)
                nc.vector.tensor_copy(osb[:, et * 512:(et + 1) * 512], ps[:])
                if last and tc4 == 3:
                    nc.sync.dma_start(
                        out[q0 + tc4 * P:q0 + (tc4 + 1) * P, et * 512:(et + 1) * 512],
                        osb[:, et * 512:(et + 1) * 512])
            if not (last and tc4 == 3):
                nc.sync.dma_start(out[q0 + tc4 * P:q0 + (tc4 + 1) * P, :], osb[:])
            pump(2)

    def attn_wo(qt):
        atts = [None] * HPC
        for h in range(HPC):
            attn_head(h, qt, atts)
        wo_block(qt, atts)

    # ---- interleaved schedule ----
    proj_block(0)
    proj_block(1)
    drain(0)
    attn_wo(0)
    proj_block(2)
    drain(1)
    attn_wo(1)
    proj_block(3)
    drain(2)
    attn_wo(2)
    drain(3)
    attn_wo(3)


_NC_CACHE = None


def _single_act_table(nc):
    """Make every activation resolve to the one table set that holds exp,
    ln AND copy (natural_log_exp_and_others). The stock assignment maps
    each function to the FIRST containing set (exp->0, ln->5), emitting an
    alternating 1.28us ACT_TABLE_LOAD per rsqrt<->softmax switch — dozens
    per kernel. Emptying the other sets (indices preserved, so the BIR
    set-id still matches act_info.json) collapses it to one load."""
    import types
    from concourse.hw_specs import get_activation_tables

    orig = get_activation_tables(nc.m.arch)
    keep = "natural_log_exp_and_others"
    assert keep in orig, sorted(orig)
    filtered = {n: (fns if n == keep else set()) for n, fns in orig.items()}

    def patched(self):
        has_activation = any(
            isinstance(i, mybir.InstActivation)
            for b in self.main_func.blocks
            for i in b.instructions
        )
        if not has_activation:
            return
        import bass_rust as _bass_rust
        _bass_rust.insert_act_table_loads(self, list(filtered.items()))

    nc.insert_act_table_loads = types.MethodType(patched, nc)


def build_nc():
    global _NC_CACHE
    if _NC_CACHE is not None:
        return _NC_CACHE
    nc = bacc.Bacc(None, target_bir_lowering=False)
    _single_act_table(nc)
    xt = nc.dram_tensor("xt", [P, 16, S], BF, kind="ExternalInput")
    wqkv = nc.dram_tensor("wqkv", [P, 16, 768], BF, kind="ExternalInput")
    wo = nc.dram_tensor("wo", [P, HPC * D], BF, kind="ExternalInput")
    cossin = nc.dram_tensor("cossin", [P, 2 * S + P], BF, kind="ExternalInput")
    gqk = nc.dram_tensor("gqk", [P, 2], F32, kind="ExternalInput")
    out = nc.dram_tensor("out", [S, D], BF, kind="ExternalOutput")
    with tile.TileContext(nc) as tc:
        with ExitStack() as ctx:
            _body(ctx, tc, xt[:], wqkv[:], wo[:], cossin[:], gqk[:], out[:])
    nc.compile()
    _NC_CACHE = nc
    return nc


def _host_tables():
    pos = np.arange(S, dtype=np.float64)
    inv_freq = 1.0 / (ROPE_THETA ** (np.arange(0, DH, 2, dtype=np.float64) / DH))
    ang = pos[:, None] * inv_freq[None, :]  # [S, 64]
    cos_s = np.concatenate([np.cos(ang), np.cos(ang)], axis=-1)  # [S, 128]
    sin_s = np.concatenate([np.sin(ang), np.sin(ang)], axis=-1)
    cos_full = np.ascontiguousarray(cos_s.T)  # [128, S]
    sins = sin_s.T.copy()
    sins[0:64] *= -1.0  # rotation sign baked in
    j = np.arange(P)[:, None]
    i = np.arange(P)[None, :]
    masktri = np.where(j <= i, 0.0, -30000.0)  # [keys, queries]
    # one [128, 2S+128] blob: [cos | sins | mask] — a single preamble DMA
    cossin = np.concatenate([cos_full, sins, masktri], axis=1).astype(BFNP)
    return cossin


def kernel(qkv, Wq, Wk, Wv, Wo, q_gamma, k_gamma):
    qkv = np.asarray(qkv, dtype=np.float32)
    Wq = np.asarray(Wq, dtype=np.float32)
    Wk = np.asarray(Wk, dtype=np.float32)
    Wv = np.asarray(Wv, dtype=np.float32)
    Wo = np.asarray(Wo, dtype=np.float32)
    q_gamma = np.asarray(q_gamma, dtype=np.float32)
    k_gamma = np.asarray(k_gamma, dtype=np.float32)

    nc = build_nc()
    cossin = _host_tables()
    gqk = np.ascontiguousarray(
        np.stack([q_gamma, k_gamma], axis=1)).astype(np.float32)  # [128, 2]
    # x^T tiles in [p, k, s] layout: element [p, k, s] = qkv[b].T[128k+p, s]
    xts = [
        np.ascontiguousarray(
            qkv[b].T.reshape(16, P, S).transpose(1, 0, 2)
        ).astype(BFNP)
        for b in range(B)
    ]

    in_maps = []
    for c in range(NCORES):
        b, g = c // 4, c % 4
        wq_c = Wq[4 * g * DH:(4 * g + 4) * DH, :]  # [512, D]
        wk_c = Wk[g * DH:(g + 1) * DH, :]  # [128, D]
        wv_c = Wv[g * DH:(g + 1) * DH, :]
        wqkv_c = np.concatenate([wq_c, wk_c, wv_c], axis=0).T  # [D, 768]
        wqkv_c = np.ascontiguousarray(
            wqkv_c.reshape(16, P, 768).transpose(1, 0, 2)).astype(BFNP)  # [128,16,768]
        wo_c = np.stack(
            [np.ascontiguousarray(Wo[:, (4 * g + h) * DH:(4 * g + h + 1) * DH].T)
             for h in range(HPC)]
        )  # [4, 128, D]
        wo_c = np.ascontiguousarray(
            wo_c.transpose(1, 0, 2).reshape(P, HPC * D)).astype(BFNP)
        in_maps.append({
            "xt": xts[b], "wqkv": wqkv_c, "wo": wo_c,
            "cossin": cossin, "gqk": gqk,
        })

    res = run_bass_kernel_spmd(nc, in_maps, core_ids=list(range(NCORES)))
    full = np.empty((B, S, D), np.float32)
    for b in range(B):
        acc = res.results[4 * b]["out"].astype(np.float32)
        for g in range(1, 4):
            acc += res.results[4 * b + g]["out"].astype(np.float32)
        full[b] = acc
    return full


# revision 24
# speedup vs baseline: 1.1488x; 1.0004x over previous
"""Causal GQA multi-head attention (RMSNorm-QK + RoPE) on 8 Trainium2 cores.

Sharding: (batch, kv-group). Core c owns batch c//4 and GQA group c%4,
i.e. 4 q heads + 1 kv head for one batch of 2048 tokens. Each core emits
a partial [S, D] output (row-sharded Wo); the host sums 4 partials/batch.

v1 schedule (single interleaved PE stream, ~97% target occupancy):
  proj0, proj1, attn0+wo0, proj2, attn1+wo1, proj3, attn2+wo2, attn3+wo3
with per-block epilogue work queues (rmsnorm+rope) pumped into the gaps
of the following segments so the PE never waits on the DVE/scalar chains.

Key mechanics:
  - DMA issue parallelized across engine queues (sync: x tiles, scalar:
    wqkv tiles, gpsimd: bulk prefetch of x blocks 1-3 / wo / tables);
    each dma_start costs ~0.6us of issue time on its queue, so the old
    single-queue preamble serialized ~25us of issue.
  - replicated-rsqrt: the ones-matmul sumsq psum is already broadcast
    across all 128 partitions, so sqrt/recip run on the full [128,512]
    (both are free-size-bound; same cost as a [1,512] row) and the
    row-extract + cast + PE broadcast matmuls disappear.
  - softmax normalize: reciprocal of the (replicated) rowsum psum, then
    one DVE multiply straight out of the att psum. No PE broadcast.
  - Wo evictions run on the otherwise-idle gpsimd engine.
"""

import sys

sys.path.insert(0, "/opt/trn_rl_repo")

from collections import deque
from contextlib import ExitStack

import ml_dtypes
import numpy as np

import concourse.bass as bass
import concourse.tile as tile
from concourse import bacc, mybir
from concourse.bass_utils import run_bass_kernel_spmd
from concourse.masks import make_identity

B, S, D = 2, 2048, 2048
H, HKV, DH = 16, 4, 128
P = 128
NCORES = 8
HPC = 4  # q heads per core
EPS = 1e-6
ROPE_THETA = 10000.0
BF = mybir.dt.bfloat16
F32 = mybir.dt.float32
BFNP = ml_dtypes.bfloat16

Copy = mybir.ActivationFunctionType.Copy
Exp = mybir.ActivationFunctionType.Exp
Ln = mybir.ActivationFunctionType.Ln
MULT = mybir.AluOpType.mult
ADD = mybir.AluOpType.add

NBLK = 4  # 512-token blocks
BLK = S // NBLK


def _body(ctx: ExitStack, tc: tile.TileContext, xt, wqkv, wo, cossin, gqk, out):
    nc = tc.nc

    const = ctx.enter_context(tc.tile_pool(name="const", bufs=1))
    res = ctx.enter_context(tc.tile_pool(name="res", bufs=1))
    sq_pool = ctx.enter_context(tc.tile_pool(name="sqp", bufs=3))
    srt_pool = ctx.enter_context(tc.tile_pool(name="srt", bufs=2))
    rs_pool = ctx.enter_context(tc.tile_pool(name="rsp", bufs=3))
    rope_pool = ctx.enter_context(tc.tile_pool(name="rop", bufs=2))
    exp_pool = ctx.enter_context(tc.tile_pool(name="exq", bufs=4))
    nrm_pool = ctx.enter_context(tc.tile_pool(name="nrm", bufs=2))
    att_pool = ctx.enter_context(tc.tile_pool(name="attp", bufs=2))
    osb_pool = ctx.enter_context(tc.tile_pool(name="osb", bufs=2))
    # PSUM: 8 banks = scw(2x2) + attps(1) + sumps(1) + pp(2)
    scw = ctx.enter_context(tc.tile_pool(name="scw", bufs=2, space="PSUM"))
    attps = ctx.enter_context(tc.tile_pool(name="atps", bufs=1, space="PSUM"))
    sumps = ctx.enter_context(tc.tile_pool(name="smps", bufs=1, space="PSUM"))
    pp = ctx.enter_context(tc.tile_pool(name="pp", bufs=2, space="PSUM"))

    # ---- constants / resident weights ----
    ones_sq = const.tile([P, P], BF, name="ones", tag="ones")
    nc.vector.memset(ones_sq[:], 1.0)
    ident = const.tile([P, P], BF, name="ident", tag="ident")
    make_identity(nc, ident[:])
    cossin_t = const.tile([P, 2 * S + P], BF, name="cossin", tag="cossin")
    cos_t = cossin_t[:, 0:S]
    sins_t = cossin_t[:, S:2 * S]
    mask_t = cossin_t[:, 2 * S:2 * S + P]
    gqk_t = const.tile([P, 2], F32, name="gqk", tag="gqk")
    epsq_t = const.tile([P, 1], F32, name="epsq", tag="epsq")
    nc.vector.memset(epsq_t[:], P * EPS)
    epsk_t = const.tile([P, 1], F32, name="epsk", tag="epsk")
    nc.vector.memset(epsk_t[:], EPS)

    wqkv_sb = const.tile([P, 16 * 768], BF, name="wqkv", tag="wqkv")
    wo_sb = const.tile([P, HPC * D], BF, name="wo", tag="wo")
    xt0_sb = const.tile([P, 16 * BLK], BF, name="xt0", tag="xt0")
    xtr_sb = const.tile([P, 16 * 3 * BLK], BF, name="xtr", tag="xtr")

    # resident activations, [dh, token] layouts
    qT = [res.tile([P, S], BF, name=f"qT{h}", tag=f"qT{h}") for h in range(HPC)]
    kT = res.tile([P, S], BF, name="kT", tag="kT")
    vT = res.tile([P, S], BF, name="vT", tag="vT")
    v_kd = res.tile([P, S], BF, name="vkd", tag="vkd")  # [keys, dh] chunks

    # ---- preamble DMAs ----
    # Two constraints: (a) only 8 HWDGE semaphores exist, so more
    # in-flight DMAs than that forces sem-reuse ordering waits that stall
    # the issue queues; (b) the DMA engines round-robin across queues, so
    # bulk prefetch on a parallel queue starves the urgent proj0 feeds.
    # Everything bulk goes on ONE queue (sync) in priority order; only the
    # small tables ride a second queue.
    for ka, kb in ((0, 1), (1, 2), (2, 5), (5, 9)):
        nc.sync.dma_start(wqkv_sb[:, ka * 768:kb * 768], wqkv[:, ka:kb, :])
        nc.sync.dma_start(xt0_sb[:, ka * BLK:kb * BLK], xt[:, ka:kb, 0:BLK])
    # tables ride mid-queue: needed by the first rope (~+28us) but must
    # not compete with the first k-tiles on the wire
    nc.sync.dma_start(cossin_t[:], cossin[:])
    nc.sync.dma_start(gqk_t[:], gqk[:])
    nc.sync.dma_start(wqkv_sb[:, 9 * 768:16 * 768], wqkv[:, 9:16, :])
    nc.sync.dma_start(xt0_sb[:, 9 * BLK:16 * BLK], xt[:, 9:16, 0:BLK])
    nc.sync.dma_start(xtr_sb[:, 0:16 * BLK], xt[:, :, BLK:2 * BLK])
    nc.sync.dma_start(
        xtr_sb[:, 16 * BLK:2 * 16 * BLK], xt[:, :, 2 * BLK:3 * BLK])
    nc.sync.dma_start(wo_sb[:], wo[:])
    nc.sync.dma_start(
        xtr_sb[:, 2 * 16 * BLK:3 * 16 * BLK], xt[:, :, 3 * BLK:4 * BLK])

    def xtile(nb, k):
        if nb == 0:
            return xt0_sb[:, k * BLK:(k + 1) * BLK]
        return xtr_sb[:, ((nb - 1) * 16 + k) * BLK:((nb - 1) * 16 + k + 1) * BLK]

    def wtile(k, m):
        return wqkv_sb[:, k * 768 + m * P:k * 768 + (m + 1) * P]

    # ---- per-block epilogue queues (rmsnorm + rope), pumped into gaps ----
    EPI = [deque() for _ in range(NBLK)]

    def pump(n=1):
        for _ in range(n):
            for nb in range(NBLK):
                if EPI[nb]:
                    nxt = EPI[nb].popleft()()
                    if nxt is not None:
                        EPI[nb].append(nxt)
                    break
            else:
                return

    def drain(nb):
        while EPI[nb]:
            nxt = EPI[nb].popleft()()
            if nxt is not None:
                EPI[nb].append(nxt)

    def rope_tile(dst, cols, rsf):
        """dst = (dst*cos + rot(dst)*sin) * rsf, in place; dst is the
        [P, BLK] column view; sins has the rotation sign baked into its
        first 64 rows."""
        t1 = rope_pool.tile([P, BLK], BF, name="t1", tag="t1")
        t2 = rope_pool.tile([P, BLK], BF, name="t2", tag="t2")
        nc.vector.tensor_copy(t2[0:64, :], dst[64:128, :])
        nc.vector.tensor_copy(t2[64:128, :], dst[0:64, :])
        nc.vector.tensor_tensor(t2[:], t2[:], sins_t[:, cols], MULT)
        nc.vector.tensor_tensor(t1[:], dst[:], cos_t[:, cols], MULT)
        nc.vector.tensor_tensor(t1[:], t1[:], t2[:], ADD)
        nc.vector.tensor_tensor(dst[:], t1[:], rsf[:], MULT)

    def stageA(nb, m, ps):
        cols = slice(nb * BLK, (nb + 1) * BLK)
        if m == 5:  # v: evict now, transpose to [keys, dh] chunks later
            nc.vector.tensor_copy(vT[:, cols], ps[:])

            def stageB_v():
                pst = pp.tile([P, BLK], BF, name="pst", tag="pp")
                for i in range(4):
                    c = nb * 4 + i
                    nc.tensor.transpose(pst[:, i * P:(i + 1) * P],
                                        vT[:, c * P:(c + 1) * P], ident[:])
                nc.scalar.copy(v_kd[:, cols], pst[:])
                return None

            EPI[nb].append(stageB_v)
            return
        if m < 4:
            dst, gsl, eps_t, escale = qT[m], gqk_t[:, 0:1], epsq_t, 1.0
        else:
            dst, gsl, eps_t, escale = kT, gqk_t[:, 1:2], epsk_t, 1.0 / P
        nc.scalar.activation(dst[:, cols], ps[:], Copy, bias=0.0, scale=gsl)
        sq = sq_pool.tile([P, BLK], BF, name="sq", tag="sq")
        nc.vector.tensor_tensor(sq[:], dst[:, cols], dst[:, cols], MULT)

        def stageB():
            # sumsq replicated across partitions by the ones-matmul.
            # rsqrt = exp(-0.5*ln(v)): ln and exp share ONE activation
            # table set (natural_log_exp_and_others) with the attention
            # exp, so the scalar engine never reloads tables (1.28us per
            # reload, dozens of sqrt<->exp switches otherwise). Both run
            # on the replicated [128,512] (free-size bound, same cost as
            # one row) so no row-extract/broadcast is ever needed.
            psr = pp.tile([P, BLK], F32, name="psr", tag="pp")
            nc.tensor.matmul(psr[:], ones_sq[:], sq[:], start=True, stop=True,
                             skip_group_check=True)
            lg = srt_pool.tile([P, BLK], F32, name="lg", tag="srt")
            nc.scalar.activation(lg[:], psr[:], Ln, bias=eps_t[:], scale=escale)
            rsf = rs_pool.tile([P, BLK], BF, name="rsf", tag="rsf")
            nc.scalar.activation(rsf[:], lg[:], Exp, bias=0.0, scale=-0.5)

            def stageC():
                cc = slice(nb * BLK, (nb + 1) * BLK)
                rope_tile(dst[:, cc], cc, rsf)
                return None

            return stageC

        EPI[nb].append(stageB)

    def proj_block(nb):
        if nb == 0:
            # k-outer: DMA-paced warmup; uses 6 psum banks across pools
            wide = scw.tile([P, 2 * BLK], F32, name="ps", tag="scw")
            psms = [wide[:, 0:BLK], wide[:, BLK:2 * BLK]]
            psms.append(attps.tile([P, BLK], F32, name="ps", tag="attps"))
            psms.append(sumps.tile([P, BLK], F32, name="ps", tag="sumps"))
            psms.append(pp.tile([P, BLK], F32, name="ps", tag="pp"))
            psms.append(pp.tile([P, BLK], F32, name="ps", tag="pp"))
            for k in range(16):
                for m in range(6):
                    nc.tensor.matmul(
                        psms[m], wtile(k, m), xtile(0, k),
                        start=(k == 0), stop=(k == 15), skip_group_check=True,
                    )
            for m in (5, 0, 1, 2, 3, 4):  # v first: frees its pp slot early
                stageA(0, m, psms[m])
        else:
            # k-tile first: its rope unlocks attention for all 4 heads.
            # The first two m-tiles borrow the free scw buffer (2 banks)
            # so they never wait on the previous segment's pp evictions.
            wide = scw.tile([P, 2 * BLK], F32, name="psw", tag="scw")
            for mi, m in enumerate((4, 0, 1, 2, 3, 5)):
                if mi < 2:
                    ps = wide[:, mi * BLK:(mi + 1) * BLK]
                else:
                    ps = pp.tile([P, BLK], F32, name="ps", tag="pp")[:]
                for k in range(16):
                    nc.tensor.matmul(
                        ps, wtile(k, m), xtile(nb, k),
                        start=(k == 0), stop=(k == 15), skip_group_check=True,
                    )
                stageA(nb, m, ps)
                pump(2)

    # ---- attention (software-pipelined) + Wo per query block ----
    def attn_head(h, qt, atts):
        """Emit scores/exp/AV for (h, qt). Score chunks are PAIRED into a
        [128,1024] 2-bank psum tile with ONE exp per pair."""
        nkc = 4 * qt + 4
        npair = nkc // 2
        q0 = qt * BLK
        ab = {}

        def pair(p):
            ps = scw.tile([P, 2 * BLK], F32, name="psS", tag="scw")
            exs = exp_pool.tile([P, 2 * BLK], BF, name="ex", tag="ex")
            offs = []
            for j in range(2):
                kc = 2 * p + j
                off = max(0, P * kc - q0)
                offs.append(off)
                nc.tensor.matmul(
                    ps[:, j * BLK + off:(j + 1) * BLK],
                    kT[:, kc * P:(kc + 1) * P], qT[h][:, q0 + off:q0 + BLK],
                    start=True, stop=(kc < 4 * qt), skip_group_check=True,
                )
                if kc >= 4 * qt:  # diagonal block: add -30000 upper triangle
                    nc.tensor.matmul(
                        ps[:, j * BLK + off:j * BLK + off + P], ident[:], mask_t[:],
                        start=False, stop=True, skip_group_check=True,
                    )
            nc.scalar.activation(exs[:, offs[0]:], ps[:, offs[0]:], Exp)
            return p, offs, exs

        def av(p, offs, exs):
            if p == 0:
                ab["att"] = attps.tile([P, BLK], F32, name="psA", tag="attps")
                ab["sum"] = sumps.tile([P, BLK], F32, name="psB", tag="sumps")
            for j in range(2):
                kc = 2 * p + j
                off = offs[j]
                exv = exs[:, j * BLK + off:(j + 1) * BLK]
                nc.tensor.matmul(
                    ab["att"][:, off:], v_kd[:, kc * P:(kc + 1) * P], exv,
                    start=(kc == 0), stop=(kc == nkc - 1), skip_group_check=True,
                )
                nc.tensor.matmul(
                    ab["sum"][:, off:], ones_sq[:], exv,
                    start=(kc == 0), stop=(kc == nkc - 1), skip_group_check=True,
                )

        # NO pumping inside attention: the scalar engine is ~95% busy with
        # the exp stream here (1.1us exp vs 1.28us PE per pair) and the
        # DVE must run the norm promptly to free the att/sum psum banks —
        # epilogue work injected into either queue stalls the PE.
        pend = []
        for p in range(npair):
            pend.append(pair(p))
            if len(pend) > 1:
                av(*pend.pop(0))
        while pend:
            av(*pend.pop(0))

        # normalize: rowsum psum is replicated across partitions, so one
        # reciprocal + one multiply straight out of the att psum.
        rrep = nrm_pool.tile([P, BLK], F32, name="rrep", tag="rrep")
        nc.vector.reciprocal_approx_fast(rrep[:], ab["sum"][:])
        a = att_pool.tile([P, BLK], BF, name=f"att{h}", tag=f"att{h}")
        nc.vector.tensor_tensor(a[:], ab["att"][:], rrep[:], MULT)
        atts[h] = a

    def wo_block(qt, atts):
        q0 = qt * BLK
        last = qt == NBLK - 1
        for tc4 in range(4):
            osb = osb_pool.tile([P, D], BF, name="osb", tag="osb")
            for et in range(4):
                ps = pp.tile([P, 512], F32, name="pso", tag="pp")
                for h2 in range(HPC):
                    nc.tensor.matmul(
                        ps[:], atts[h2][:, tc4 * P:(tc4 + 1) * P],
                        wo_sb[:, h2 * D + et * 512:h2 * D + (et + 1) * 512],
                        start=(h2 == 0), stop=(h2 == HPC - 1), skip_group_check=True,
                    )
                # evicts on vector only: scalar must stay clear for the
                # next attention block's exp stream (gpsimd can't read
                # PSUM)
                nc.vector.tensor_copy(osb[:, et * 512:(et + 1) * 512], ps[:])
                if last and tc4 == 3:
                    nc.sync.dma_start(
                        out[q0 + tc4 * P:q0 + (tc4 + 1) * P, et * 512:(et + 1) * 512],
                        osb[:, et * 512:(et + 1) * 512])
            if not (last and tc4 == 3):
                nc.sync.dma_start(out[q0 + tc4 * P:q0 + (tc4 + 1) * P, :], osb[:])
            pump(2)

    def attn_wo(qt):
        atts = [None] * HPC
        for h in range(HPC):
            attn_head(h, qt, atts)
        wo_block(qt, atts)

    # ---- interleaved schedule ----
    proj_block(0)
    proj_block(1)
    drain(0)
    attn_wo(0)
    proj_block(2)
    drain(1)
    attn_wo(1)
    proj_block(3)
    drain(2)
    attn_wo(2)
    drain(3)
    attn_wo(3)


_NC_CACHE = None


def _single_act_table(nc):
    """Make every activation resolve to the one table set that holds exp,
    ln AND copy (natural_log_exp_and_others). The stock assignment maps
    each function to the FIRST containing set (exp->0, ln->5), emitting an
    alternating 1.28us ACT_TABLE_LOAD per rsqrt<->softmax switch — dozens
    per kernel. Emptying the other sets (indices preserved, so the BIR
    set-id still matches act_info.json) collapses it to one load."""
    import types
    from concourse.hw_specs import get_activation_tables

    orig = get_activation_tables(nc.m.arch)
    keep = "natural_log_exp_and_others"
    assert keep in orig, sorted(orig)
    filtered = {n: (fns if n == keep else set()) for n, fns in orig.items()}

    def patched(self):
        has_activation = any(
            isinstance(i, mybir.InstActivation)
            for b in self.main_func.blocks
            for i in b.instructions
        )
        if not has_activation:
            return
        import bass_rust as _bass_rust
        _bass_rust.insert_act_table_loads(self, list(filtered.items()))

    nc.insert_act_table_loads = types.MethodType(patched, nc)


def build_nc():
    global _NC_CACHE
    if _NC_CACHE is not None:
        return _NC_CACHE
    nc = bacc.Bacc(None, target_bir_lowering=False)
    _single_act_table(nc)
    xt = nc.dram_tensor("xt", [P, 16, S], BF, kind="ExternalInput")
    wqkv = nc.dram_tensor("wqkv", [P, 16, 768], BF, kind="ExternalInput")
    wo = nc.dram_tensor("wo", [P, HPC * D], BF, kind="ExternalInput")
    cossin = nc.dram_tensor("cossin", [P, 2 * S + P], BF, kind="ExternalInput")
    gqk = nc.dram_tensor("gqk", [P, 2], F32, kind="ExternalInput")
    out = nc.dram_tensor("out", [S, D], BF, kind="ExternalOutput")
    with tile.TileContext(nc) as tc:
        with ExitStack() as ctx:
            _body(ctx, tc, xt[:], wqkv[:], wo[:], cossin[:], gqk[:], out[:])
    nc.compile()
    _NC_CACHE = nc
    return nc


def _host_tables():
    pos = np.arange(S, dtype=np.float64)
    inv_freq = 1.0 / (ROPE_THETA ** (np.arange(0, DH, 2, dtype=np.float64) / DH))
    ang = pos[:, None] * inv_freq[None, :]  # [S, 64]
    cos_s = np.concatenate([np.cos(ang), np.cos(ang)], axis=-1)  # [S, 128]
    sin_s = np.concatenate([np.sin(ang), np.sin(ang)], axis=-1)
    cos_full = np.ascontiguousarray(cos_s.T)  # [128, S]
    sins = sin_s.T.copy()
    sins[0:64] *= -1.0  # rotation sign baked in
    j = np.arange(P)[:, None]
    i = np.arange(P)[None, :]
    masktri = np.where(j <= i, 0.0, -30000.0)  # [keys, queries]
    # one [128, 2S+128] blob: [cos | sins | mask] — a single preamble DMA
    cossin = np.concatenate([cos_full, sins, masktri], axis=1).astype(BFNP)
    return cossin


def kernel(qkv, Wq, Wk, Wv, Wo, q_gamma, k_gamma):
    qkv = np.asarray(qkv, dtype=np.float32)
    Wq = np.asarray(Wq, dtype=np.float32)
    Wk = np.asarray(Wk, dtype=np.float32)
    Wv = np.asarray(Wv, dtype=np.float32)
    Wo = np.asarray(Wo, dtype=np.float32)
    q_gamma = np.asarray(q_gamma, dtype=np.float32)
    k_gamma = np.asarray(k_gamma, dtype=np.float32)

    nc = build_nc()
    cossin = _host_tables()
    gqk = np.ascontiguousarray(
        np.stack([q_gamma, k_gamma], axis=1)).astype(np.float32)  # [128, 2]
    # x^T tiles in [p, k, s] layout: element [p, k, s] = qkv[b].T[128k+p, s]
    xts = [
        np.ascontiguousarray(
            qkv[b].T.reshape(16, P, S).transpose(1, 0, 2)
        ).astype(BFNP)
        for b in range(B)
    ]

    in_maps = []
    for c in range(NCORES):
        b, g = c // 4, c % 4
        wq_c = Wq[4 * g * DH:(4 * g + 4) * DH, :]  # [512, D]
        wk_c = Wk[g * DH:(g + 1) * DH, :]  # [128, D]
        wv_c = Wv[g * DH:(g + 1) * DH, :]
        wqkv_c = np.concatenate([wq_c, wk_c, wv_c], axis=0).T  # [D, 768]
        wqkv_c = np.ascontiguousarray(
            wqkv_c.reshape(16, P, 768).transpose(1, 0, 2)).astype(BFNP)  # [128,16,768]
        wo_c = np.stack(
            [np.ascontiguousarray(Wo[:, (4 * g + h) * DH:(4 * g + h + 1) * DH].T)
             for h in range(HPC)]
        )  # [4, 128, D]
        wo_c = np.ascontiguousarray(
            wo_c.transpose(1, 0, 2).reshape(P, HPC * D)).astype(BFNP)
        in_maps.append({
            "xt": xts[b], "wqkv": wqkv_c, "wo": wo_c,
            "cossin": cossin, "gqk": gqk,
        })

    res = run_bass_kernel_spmd(nc, in_maps, core_ids=list(range(NCORES)))
    full = np.empty((B, S, D), np.float32)
    for b in range(B):
        acc = res.results[4 * b]["out"].astype(np.float32)
        for g in range(1, 4):
            acc += res.results[4 * b + g]["out"].astype(np.float32)
        full[b] = acc
    return full


# revision 26
# speedup vs baseline: 1.1987x; 1.0434x over previous
"""Causal GQA multi-head attention (RMSNorm-QK + RoPE) on 8 Trainium2 cores.

Sharding: (batch, kv-group). Core c owns batch c//4 and GQA group c%4,
i.e. 4 q heads + 1 kv head for one batch of 2048 tokens. Each core emits
a partial [S, D] output (row-sharded Wo); the host sums 4 partials/batch.

v1 schedule (single interleaved PE stream, ~97% target occupancy):
  proj0, proj1, attn0+wo0, proj2, attn1+wo1, proj3, attn2+wo2, attn3+wo3
with per-block epilogue work queues (rmsnorm+rope) pumped into the gaps
of the following segments so the PE never waits on the DVE/scalar chains.

Key mechanics:
  - DMA issue parallelized across engine queues (sync: x tiles, scalar:
    wqkv tiles, gpsimd: bulk prefetch of x blocks 1-3 / wo / tables);
    each dma_start costs ~0.6us of issue time on its queue, so the old
    single-queue preamble serialized ~25us of issue.
  - replicated-rsqrt: the ones-matmul sumsq psum is already broadcast
    across all 128 partitions, so sqrt/recip run on the full [128,512]
    (both are free-size-bound; same cost as a [1,512] row) and the
    row-extract + cast + PE broadcast matmuls disappear.
  - softmax normalize: reciprocal of the (replicated) rowsum psum, then
    one DVE multiply straight out of the att psum. No PE broadcast.
  - Wo evictions run on the otherwise-idle gpsimd engine.
"""

import sys

sys.path.insert(0, "/opt/trn_rl_repo")

from collections import deque
from contextlib import ExitStack

import ml_dtypes
import numpy as np

import concourse.bass as bass
import concourse.tile as tile
from concourse import bacc, mybir
from concourse.bass_utils import run_bass_kernel_spmd
from concourse.masks import make_identity

B, S, D = 2, 2048, 2048
H, HKV, DH = 16, 4, 128
P = 128
NCORES = 8
HPC = 4  # q heads per core
EPS = 1e-6
ROPE_THETA = 10000.0
BF = mybir.dt.bfloat16
F32 = mybir.dt.float32
BFNP = ml_dtypes.bfloat16

Copy = mybir.ActivationFunctionType.Copy
Exp = mybir.ActivationFunctionType.Exp
Ln = mybir.ActivationFunctionType.Ln
MULT = mybir.AluOpType.mult
ADD = mybir.AluOpType.add

NBLK = 4  # 512-token blocks
BLK = S // NBLK


def _body(ctx: ExitStack, tc: tile.TileContext, xt, wqkv, wo, cossin, gqk, out):
    nc = tc.nc

    const = ctx.enter_context(tc.tile_pool(name="const", bufs=1))
    res = ctx.enter_context(tc.tile_pool(name="res", bufs=1))
    sq_pool = ctx.enter_context(tc.tile_pool(name="sqp", bufs=3))
    srt_pool = ctx.enter_context(tc.tile_pool(name="srt", bufs=2))
    rs_pool = ctx.enter_context(tc.tile_pool(name="rsp", bufs=3))
    rope_pool = ctx.enter_context(tc.tile_pool(name="rop", bufs=2))
    exp_pool = ctx.enter_context(tc.tile_pool(name="exq", bufs=4))
    nrm_pool = ctx.enter_context(tc.tile_pool(name="nrm", bufs=2))
    att_pool = ctx.enter_context(tc.tile_pool(name="attp", bufs=2))
    osb_pool = ctx.enter_context(tc.tile_pool(name="osb", bufs=2))
    # PSUM: 8 banks = scw(2x2) + attps(1) + sumps(1) + pp(2)
    scw = ctx.enter_context(tc.tile_pool(name="scw", bufs=2, space="PSUM"))
    attps = ctx.enter_context(tc.tile_pool(name="atps", bufs=1, space="PSUM"))
    sumps = ctx.enter_context(tc.tile_pool(name="smps", bufs=1, space="PSUM"))
    pp = ctx.enter_context(tc.tile_pool(name="pp", bufs=2, space="PSUM"))

    # ---- constants / resident weights ----
    ones_sq = const.tile([P, P], BF, name="ones", tag="ones")
    nc.vector.memset(ones_sq[:], 1.0)
    ident = const.tile([P, P], BF, name="ident", tag="ident")
    make_identity(nc, ident[:])
    cossin_t = const.tile([P, 2 * S + P], BF, name="cossin", tag="cossin")
    cos_t = cossin_t[:, 0:S]
    sins_t = cossin_t[:, S:2 * S]
    mask_t = cossin_t[:, 2 * S:2 * S + P]
    gqk_t = const.tile([P, 2], F32, name="gqk", tag="gqk")
    epsq_t = const.tile([P, 1], F32, name="epsq", tag="epsq")
    nc.vector.memset(epsq_t[:], P * EPS)
    epsk_t = const.tile([P, 1], F32, name="epsk", tag="epsk")
    nc.vector.memset(epsk_t[:], EPS)

    wqkv_sb = const.tile([P, 16 * 768], BF, name="wqkv", tag="wqkv")
    wo_sb = const.tile([P, HPC * D], BF, name="wo", tag="wo")
    xt0_sb = const.tile([P, 16 * BLK], BF, name="xt0", tag="xt0")
    xtr_sb = const.tile([P, 16 * 3 * BLK], BF, name="xtr", tag="xtr")

    # resident activations, [dh, token] layouts
    qT = [res.tile([P, S], BF, name=f"qT{h}", tag=f"qT{h}") for h in range(HPC)]
    kT = res.tile([P, S], BF, name="kT", tag="kT")
    vT = res.tile([P, S], BF, name="vT", tag="vT")
    v_kd = res.tile([P, S], BF, name="vkd", tag="vkd")  # [keys, dh] chunks

    # ---- preamble DMAs ----
    # Two constraints: (a) only 8 HWDGE semaphores exist, so more
    # in-flight DMAs than that forces sem-reuse ordering waits that stall
    # the issue queues; (b) the DMA engines round-robin across queues, so
    # bulk prefetch on a parallel queue starves the urgent proj0 feeds.
    # Everything bulk goes on ONE queue (sync) in priority order; only the
    # small tables ride a second queue.
    for ka, kb in ((0, 1), (1, 2), (2, 5), (5, 9)):
        nc.sync.dma_start(wqkv_sb[:, ka * 768:kb * 768], wqkv[:, ka:kb, :])
        nc.sync.dma_start(xt0_sb[:, ka * BLK:kb * BLK], xt[:, ka:kb, 0:BLK])
    nc.sync.dma_start(wqkv_sb[:, 9 * 768:16 * 768], wqkv[:, 9:16, :])
    nc.sync.dma_start(xt0_sb[:, 9 * BLK:16 * BLK], xt[:, 9:16, 0:BLK])
    # tables after the proj0 feeds (consumed by the first rope ~+30us)
    nc.sync.dma_start(cossin_t[:], cossin[:])
    nc.sync.dma_start(gqk_t[:], gqk[:])
    nc.sync.dma_start(xtr_sb[:, 0:16 * BLK], xt[:, :, BLK:2 * BLK])
    nc.sync.dma_start(
        xtr_sb[:, 16 * BLK:2 * 16 * BLK], xt[:, :, 2 * BLK:3 * BLK])
    nc.sync.dma_start(wo_sb[:], wo[:])
    nc.sync.dma_start(
        xtr_sb[:, 2 * 16 * BLK:3 * 16 * BLK], xt[:, :, 3 * BLK:4 * BLK])

    def xtile(nb, k):
        if nb == 0:
            return xt0_sb[:, k * BLK:(k + 1) * BLK]
        return xtr_sb[:, ((nb - 1) * 16 + k) * BLK:((nb - 1) * 16 + k + 1) * BLK]

    def wtile(k, m):
        return wqkv_sb[:, k * 768 + m * P:k * 768 + (m + 1) * P]

    # ---- per-block epilogue queues (rmsnorm + rope), pumped into gaps ----
    EPI = [deque() for _ in range(NBLK)]

    def pump(n=1):
        for _ in range(n):
            for nb in range(NBLK):
                if EPI[nb]:
                    nxt = EPI[nb].popleft()()
                    if nxt is not None:
                        EPI[nb].append(nxt)
                    break
            else:
                return

    def drain(nb):
        while EPI[nb]:
            nxt = EPI[nb].popleft()()
            if nxt is not None:
                EPI[nb].append(nxt)

    def rope_tile(dst, cols, rsf):
        """dst = (dst*cos + rot(dst)*sin) * rsf, in place; dst is the
        [P, BLK] column view; sins has the rotation sign baked into its
        first 64 rows."""
        t1 = rope_pool.tile([P, BLK], BF, name="t1", tag="t1")
        t2 = rope_pool.tile([P, BLK], BF, name="t2", tag="t2")
        nc.vector.tensor_copy(t2[0:64, :], dst[64:128, :])
        nc.vector.tensor_copy(t2[64:128, :], dst[0:64, :])
        nc.vector.tensor_tensor(t2[:], t2[:], sins_t[:, cols], MULT)
        nc.vector.tensor_tensor(t1[:], dst[:], cos_t[:, cols], MULT)
        nc.vector.tensor_tensor(t1[:], t1[:], t2[:], ADD)
        nc.vector.tensor_tensor(dst[:], t1[:], rsf[:], MULT)

    def stageA(nb, m, ps):
        cols = slice(nb * BLK, (nb + 1) * BLK)
        if m == 5:  # v: evict now, transpose to [keys, dh] chunks later
            nc.vector.tensor_copy(vT[:, cols], ps[:])

            def stageB_v():
                pst = pp.tile([P, BLK], BF, name="pst", tag="pp")
                for i in range(4):
                    c = nb * 4 + i
                    nc.tensor.transpose(pst[:, i * P:(i + 1) * P],
                                        vT[:, c * P:(c + 1) * P], ident[:])
                nc.scalar.copy(v_kd[:, cols], pst[:])
                return None

            EPI[nb].append(stageB_v)
            return
        if m < 4:
            dst, gsl, eps_t, escale = qT[m], gqk_t[:, 0:1], epsq_t, 1.0
        else:
            dst, gsl, eps_t, escale = kT, gqk_t[:, 1:2], epsk_t, 1.0 / P
        nc.scalar.activation(dst[:, cols], ps[:], Copy, bias=0.0, scale=gsl)
        sq = sq_pool.tile([P, BLK], BF, name="sq", tag="sq")
        nc.vector.tensor_tensor(sq[:], dst[:, cols], dst[:, cols], MULT)

        def stageB():
            # sumsq replicated across partitions by the ones-matmul.
            # rsqrt = exp(-0.5*ln(v)): ln and exp share ONE activation
            # table set (natural_log_exp_and_others) with the attention
            # exp, so the scalar engine never reloads tables (1.28us per
            # reload, dozens of sqrt<->exp switches otherwise). Both run
            # on the replicated [128,512] (free-size bound, same cost as
            # one row) so no row-extract/broadcast is ever needed.
            psr = pp.tile([P, BLK], F32, name="psr", tag="pp")
            nc.tensor.matmul(psr[:], ones_sq[:], sq[:], start=True, stop=True,
                             skip_group_check=True)
            lg = srt_pool.tile([P, BLK], F32, name="lg", tag="srt")
            nc.scalar.activation(lg[:], psr[:], Ln, bias=eps_t[:], scale=escale)
            rsf = rs_pool.tile([P, BLK], BF, name="rsf", tag="rsf")
            nc.scalar.activation(rsf[:], lg[:], Exp, bias=0.0, scale=-0.5)

            def stageC():
                cc = slice(nb * BLK, (nb + 1) * BLK)
                rope_tile(dst[:, cc], cc, rsf)
                return None

            return stageC

        EPI[nb].append(stageB)

    def proj_block(nb):
        if nb == 0:
            # k-outer: DMA-paced warmup; uses 6 psum banks across pools
            wide = scw.tile([P, 2 * BLK], F32, name="ps", tag="scw")
            psms = [wide[:, 0:BLK], wide[:, BLK:2 * BLK]]
            psms.append(attps.tile([P, BLK], F32, name="ps", tag="attps"))
            psms.append(sumps.tile([P, BLK], F32, name="ps", tag="sumps"))
            psms.append(pp.tile([P, BLK], F32, name="ps", tag="pp"))
            psms.append(pp.tile([P, BLK], F32, name="ps", tag="pp"))
            for k in range(16):
                for m in range(6):
                    nc.tensor.matmul(
                        psms[m], wtile(k, m), xtile(0, k),
                        start=(k == 0), stop=(k == 15), skip_group_check=True,
                    )
            for m in (5, 0, 1, 2, 3, 4):  # v first: frees its pp slot early
                stageA(0, m, psms[m])
        else:
            # k-tile first: its rope unlocks attention for all 4 heads.
            # The first two m-tiles borrow the free scw buffer (2 banks)
            # so they never wait on the previous segment's pp evictions.
            wide = scw.tile([P, 2 * BLK], F32, name="psw", tag="scw")
            for mi, m in enumerate((4, 0, 1, 2, 3, 5)):
                if mi < 2:
                    ps = wide[:, mi * BLK:(mi + 1) * BLK]
                else:
                    ps = pp.tile([P, BLK], F32, name="ps", tag="pp")[:]
                for k in range(16):
                    nc.tensor.matmul(
                        ps, wtile(k, m), xtile(nb, k),
                        start=(k == 0), stop=(k == 15), skip_group_check=True,
                    )
                stageA(nb, m, ps)
                pump(2)

    # ---- attention (software-pipelined) + Wo per query block ----
    def attn_head(h, qt, atts):
        """Emit scores/exp/AV for (h, qt). Score chunks are PAIRED into a
        [128,1024] 2-bank psum tile with ONE exp per pair."""
        nkc = 4 * qt + 4
        npair = nkc // 2
        q0 = qt * BLK
        ab = {}

        def pair(p):
            ps = scw.tile([P, 2 * BLK], F32, name="psS", tag="scw")
            exs = exp_pool.tile([P, 2 * BLK], BF, name="ex", tag="ex")
            offs = []
            for j in range(2):
                kc = 2 * p + j
                off = max(0, P * kc - q0)
                offs.append(off)
                nc.tensor.matmul(
                    ps[:, j * BLK + off:(j + 1) * BLK],
                    kT[:, kc * P:(kc + 1) * P], qT[h][:, q0 + off:q0 + BLK],
                    start=True, stop=(kc < 4 * qt), skip_group_check=True,
                )
                if kc >= 4 * qt:  # diagonal block: add -30000 upper triangle
                    nc.tensor.matmul(
                        ps[:, j * BLK + off:j * BLK + off + P], ident[:], mask_t[:],
                        start=False, stop=True, skip_group_check=True,
                    )
            nc.scalar.activation(exs[:, offs[0]:], ps[:, offs[0]:], Exp)
            return p, offs, exs

        def av(p, offs, exs):
            if p == 0:
                ab["att"] = attps.tile([P, BLK], F32, name="psA", tag="attps")
                ab["sum"] = sumps.tile([P, BLK], F32, name="psB", tag="sumps")
            for j in range(2):
                kc = 2 * p + j
                off = offs[j]
                exv = exs[:, j * BLK + off:(j + 1) * BLK]
                nc.tensor.matmul(
                    ab["att"][:, off:], v_kd[:, kc * P:(kc + 1) * P], exv,
                    start=(kc == 0), stop=(kc == nkc - 1), skip_group_check=True,
                )
                nc.tensor.matmul(
                    ab["sum"][:, off:], ones_sq[:], exv,
                    start=(kc == 0), stop=(kc == nkc - 1), skip_group_check=True,
                )

        # NO pumping inside attention: the scalar engine is ~95% busy with
        # the exp stream here (1.1us exp vs 1.28us PE per pair) and the
        # DVE must run the norm promptly to free the att/sum psum banks —
        # epilogue work injected into either queue stalls the PE.
        pend = []
        for p in range(npair):
            pend.append(pair(p))
            if len(pend) > 1:
                av(*pend.pop(0))
        while pend:
            av(*pend.pop(0))

        # normalize: rowsum psum is replicated across partitions, so one
        # reciprocal + one multiply straight out of the att psum.
        rrep = nrm_pool.tile([P, BLK], F32, name="rrep", tag="rrep")
        nc.vector.reciprocal_approx_fast(rrep[:], ab["sum"][:])
        a = att_pool.tile([P, BLK], BF, name=f"att{h}", tag=f"att{h}")
        nc.vector.tensor_tensor(a[:], ab["att"][:], rrep[:], MULT)
        atts[h] = a

    def wo_tc4(qt, tc4, atts, tail=False):
        """One 128-query group of the Wo projection for query block qt.
        Interleaved between attention heads of block qt+1: the ~3.4us of
        exp-free PE work absorbs the previous head's norm latency and
        gives the scalar engine slack for pumped epilogue work."""
        q0 = qt * BLK
        osb = osb_pool.tile([P, D], BF, name="osb", tag="osb")
        for et in range(4):
            ps = pp.tile([P, 512], F32, name="pso", tag="pp")
            for h2 in range(HPC):
                nc.tensor.matmul(
                    ps[:], atts[h2][:, tc4 * P:(tc4 + 1) * P],
                    wo_sb[:, h2 * D + et * 512:h2 * D + (et + 1) * 512],
                    start=(h2 == 0), stop=(h2 == HPC - 1), skip_group_check=True,
                )
            # evicts on vector only: scalar must stay clear for the exp
            # stream (gpsimd can't read PSUM)
            nc.vector.tensor_copy(osb[:, et * 512:(et + 1) * 512], ps[:])
            if tail:
                nc.sync.dma_start(
                    out[q0 + tc4 * P:q0 + (tc4 + 1) * P, et * 512:(et + 1) * 512],
                    osb[:, et * 512:(et + 1) * 512])
        if not tail:
            nc.sync.dma_start(out[q0 + tc4 * P:q0 + (tc4 + 1) * P, :], osb[:])
        pump(2)

    def attn_block(qt, prev_atts):
        atts = [None] * HPC
        for h in range(HPC):
            attn_head(h, qt, atts)
            if prev_atts is not None:
                wo_tc4(qt - 1, h, prev_atts)
        return atts

    # ---- interleaved schedule ----
    proj_block(0)
    proj_block(1)
    drain(0)
    atts0 = attn_block(0, None)
    proj_block(2)
    drain(1)
    atts1 = attn_block(1, atts0)
    proj_block(3)
    drain(2)
    atts2 = attn_block(2, atts1)
    drain(3)
    atts3 = attn_block(3, atts2)
    for tc4 in range(4):
        wo_tc4(3, tc4, atts3, tail=(tc4 == 3))


_NC_CACHE = None


def _single_act_table(nc):
    """Make every activation resolve to the one table set that holds exp,
    ln AND copy (natural_log_exp_and_others). The stock assignment maps
    each function to the FIRST containing set (exp->0, ln->5), emitting an
    alternating 1.28us ACT_TABLE_LOAD per rsqrt<->softmax switch — dozens
    per kernel. Emptying the other sets (indices preserved, so the BIR
    set-id still matches act_info.json) collapses it to one load."""
    import types
    from concourse.hw_specs import get_activation_tables

    orig = get_activation_tables(nc.m.arch)
    keep = "natural_log_exp_and_others"
    assert keep in orig, sorted(orig)
    filtered = {n: (fns if n == keep else set()) for n, fns in orig.items()}

    def patched(self):
        has_activation = any(
            isinstance(i, mybir.InstActivation)
            for b in self.main_func.blocks
            for i in b.instructions
        )
        if not has_activation:
            return
        import bass_rust as _bass_rust
        _bass_rust.insert_act_table_loads(self, list(filtered.items()))

    nc.insert_act_table_loads = types.MethodType(patched, nc)


def build_nc():
    global _NC_CACHE
    if _NC_CACHE is not None:
        return _NC_CACHE
    nc = bacc.Bacc(None, target_bir_lowering=False)
    _single_act_table(nc)
    xt = nc.dram_tensor("xt", [P, 16, S], BF, kind="ExternalInput")
    wqkv = nc.dram_tensor("wqkv", [P, 16, 768], BF, kind="ExternalInput")
    wo = nc.dram_tensor("wo", [P, HPC * D], BF, kind="ExternalInput")
    cossin = nc.dram_tensor("cossin", [P, 2 * S + P], BF, kind="ExternalInput")
    gqk = nc.dram_tensor("gqk", [P, 2], F32, kind="ExternalInput")
    out = nc.dram_tensor("out", [S, D], BF, kind="ExternalOutput")
    with tile.TileContext(nc) as tc:
        with ExitStack() as ctx:
            _body(ctx, tc, xt[:], wqkv[:], wo[:], cossin[:], gqk[:], out[:])
    nc.compile()
    _NC_CACHE = nc
    return nc


def _host_tables():
    pos = np.arange(S, dtype=np.float64)
    inv_freq = 1.0 / (ROPE_THETA ** (np.arange(0, DH, 2, dtype=np.float64) / DH))
    ang = pos[:, None] * inv_freq[None, :]  # [S, 64]
    cos_s = np.concatenate([np.cos(ang), np.cos(ang)], axis=-1)  # [S, 128]
    sin_s = np.concatenate([np.sin(ang), np.sin(ang)], axis=-1)
    cos_full = np.ascontiguousarray(cos_s.T)  # [128, S]
    sins = sin_s.T.copy()
    sins[0:64] *= -1.0  # rotation sign baked in
    j = np.arange(P)[:, None]
    i = np.arange(P)[None, :]
    masktri = np.where(j <= i, 0.0, -30000.0)  # [keys, queries]
    # one [128, 2S+128] blob: [cos | sins | mask] — a single preamble DMA
    cossin = np.concatenate([cos_full, sins, masktri], axis=1).astype(BFNP)
    return cossin


def kernel(qkv, Wq, Wk, Wv, Wo, q_gamma, k_gamma):
    qkv = np.asarray(qkv, dtype=np.float32)
    Wq = np.asarray(Wq, dtype=np.float32)
    Wk = np.asarray(Wk, dtype=np.float32)
    Wv = np.asarray(Wv, dtype=np.float32)
    Wo = np.asarray(Wo, dtype=np.float32)
    q_gamma = np.asarray(q_gamma, dtype=np.float32)
    k_gamma = np.asarray(k_gamma, dtype=np.float32)

    nc = build_nc()
    cossin = _host_tables()
    gqk = np.ascontiguousarray(
        np.stack([q_gamma, k_gamma], axis=1)).astype(np.float32)  # [128, 2]
    # x^T tiles in [p, k, s] layout: element [p, k, s] = qkv[b].T[128k+p, s]
    xts = [
        np.ascontiguousarray(
            qkv[b].T.reshape(16, P, S).transpose(1, 0, 2)
        ).astype(BFNP)
        for b in range(B)
    ]

    in_maps = []
    for c in range(NCORES):
        b, g = c // 4, c % 4
        wq_c = Wq[4 * g * DH:(4 * g + 4) * DH, :]  # [512, D]
        wk_c = Wk[g * DH:(g + 1) * DH, :]  # [128, D]
        wv_c = Wv[g * DH:(g + 1) * DH, :]
        wqkv_c = np.concatenate([wq_c, wk_c, wv_c], axis=0).T  # [D, 768]
        wqkv_c = np.ascontiguousarray(
            wqkv_c.reshape(16, P, 768).transpose(1, 0, 2)).astype(BFNP)  # [128,16,768]
        wo_c = np.stack(
            [np.ascontiguousarray(Wo[:, (4 * g + h) * DH:(4 * g + h + 1) * DH].T)
             for h in range(HPC)]
        )  # [4, 128, D]
        wo_c = np.ascontiguousarray(
            wo_c.transpose(1, 0, 2).reshape(P, HPC * D)).astype(BFNP)
        in_maps.append({
            "xt": xts[b], "wqkv": wqkv_c, "wo": wo_c,
            "cossin": cossin, "gqk": gqk,
        })

    res = run_bass_kernel_spmd(nc, in_maps, core_ids=list(range(NCORES)))
    full = np.empty((B, S, D), np.float32)
    for b in range(B):
        acc = res.results[4 * b]["out"].astype(np.float32)
        for g in range(1, 4):
            acc += res.results[4 * b + g]["out"].astype(np.float32)
        full[b] = acc
    return full


# revision 28
# speedup vs baseline: 1.2161x; 1.0146x over previous
"""Causal GQA multi-head attention (RMSNorm-QK + RoPE) on 8 Trainium2 cores.

Sharding: (batch, kv-group). Core c owns batch c//4 and GQA group c%4,
i.e. 4 q heads + 1 kv head for one batch of 2048 tokens. Each core emits
a partial [S, D] output (row-sharded Wo); the host sums 4 partials/batch.

v1 schedule (single interleaved PE stream, ~97% target occupancy):
  proj0, proj1, attn0+wo0, proj2, attn1+wo1, proj3, attn2+wo2, attn3+wo3
with per-block epilogue work queues (rmsnorm+rope) pumped into the gaps
of the following segments so the PE never waits on the DVE/scalar chains.

Key mechanics:
  - DMA issue parallelized across engine queues (sync: x tiles, scalar:
    wqkv tiles, gpsimd: bulk prefetch of x blocks 1-3 / wo / tables);
    each dma_start costs ~0.6us of issue time on its queue, so the old
    single-queue preamble serialized ~25us of issue.
  - replicated-rsqrt: the ones-matmul sumsq psum is already broadcast
    across all 128 partitions, so sqrt/recip run on the full [128,512]
    (both are free-size-bound; same cost as a [1,512] row) and the
    row-extract + cast + PE broadcast matmuls disappear.
  - softmax normalize: reciprocal of the (replicated) rowsum psum, then
    one DVE multiply straight out of the att psum. No PE broadcast.
  - Wo evictions run on the otherwise-idle gpsimd engine.
"""

import sys

sys.path.insert(0, "/opt/trn_rl_repo")

from collections import deque
from contextlib import ExitStack

import ml_dtypes
import numpy as np

import concourse.bass as bass
import concourse.tile as tile
from concourse import bacc, mybir
from concourse.bass_utils import run_bass_kernel_spmd
from concourse.masks import make_identity

B, S, D = 2, 2048, 2048
H, HKV, DH = 16, 4, 128
P = 128
NCORES = 8
HPC = 4  # q heads per core
EPS = 1e-6
ROPE_THETA = 10000.0
BF = mybir.dt.bfloat16
F32 = mybir.dt.float32
BFNP = ml_dtypes.bfloat16

Copy = mybir.ActivationFunctionType.Copy
Exp = mybir.ActivationFunctionType.Exp
Ln = mybir.ActivationFunctionType.Ln
MULT = mybir.AluOpType.mult
ADD = mybir.AluOpType.add

NBLK = 4  # 512-token blocks
BLK = S // NBLK


def _body(ctx: ExitStack, tc: tile.TileContext, xt, wqkv, wo, cossin, gqk, out):
    nc = tc.nc

    const = ctx.enter_context(tc.tile_pool(name="const", bufs=1))
    res = ctx.enter_context(tc.tile_pool(name="res", bufs=1))
    sq_pool = ctx.enter_context(tc.tile_pool(name="sqp", bufs=3))
    srt_pool = ctx.enter_context(tc.tile_pool(name="srt", bufs=2))
    rs_pool = ctx.enter_context(tc.tile_pool(name="rsp", bufs=3))
    rope_pool = ctx.enter_context(tc.tile_pool(name="rop", bufs=2))
    exp_pool = ctx.enter_context(tc.tile_pool(name="exq", bufs=4))
    nrm_pool = ctx.enter_context(tc.tile_pool(name="nrm", bufs=2))
    att_pool = ctx.enter_context(tc.tile_pool(name="attp", bufs=2))
    osb_pool = ctx.enter_context(tc.tile_pool(name="osb", bufs=2))
    # PSUM: 8 banks = scw(2x2) + attps(1) + sumps(1) + pp(2)
    scw = ctx.enter_context(tc.tile_pool(name="scw", bufs=2, space="PSUM"))
    attps = ctx.enter_context(tc.tile_pool(name="atps", bufs=1, space="PSUM"))
    sumps = ctx.enter_context(tc.tile_pool(name="smps", bufs=1, space="PSUM"))
    pp = ctx.enter_context(tc.tile_pool(name="pp", bufs=2, space="PSUM"))

    # ---- constants / resident weights ----
    ones_sq = const.tile([P, P], BF, name="ones", tag="ones")
    nc.vector.memset(ones_sq[:], 1.0)
    ident = const.tile([P, P], BF, name="ident", tag="ident")
    make_identity(nc, ident[:])
    cossin_t = const.tile([P, 2 * S + P], BF, name="cossin", tag="cossin")
    cos_t = cossin_t[:, 0:S]
    sins_t = cossin_t[:, S:2 * S]
    mask_t = cossin_t[:, 2 * S:2 * S + P]
    gqk_t = const.tile([P, 2], F32, name="gqk", tag="gqk")
    epsq_t = const.tile([P, 1], F32, name="epsq", tag="epsq")
    nc.vector.memset(epsq_t[:], P * EPS)
    epsk_t = const.tile([P, 1], F32, name="epsk", tag="epsk")
    nc.vector.memset(epsk_t[:], EPS)

    wqkv_sb = const.tile([P, 16 * 768], BF, name="wqkv", tag="wqkv")
    wo_sb = const.tile([P, HPC * D], BF, name="wo", tag="wo")
    xt0_sb = const.tile([P, 16 * BLK], BF, name="xt0", tag="xt0")
    xtr_sb = const.tile([P, 16 * 3 * BLK], BF, name="xtr", tag="xtr")

    # resident activations, [dh, token] layouts
    qT = [res.tile([P, S], BF, name=f"qT{h}", tag=f"qT{h}") for h in range(HPC)]
    kT = res.tile([P, S], BF, name="kT", tag="kT")
    vT = res.tile([P, S], BF, name="vT", tag="vT")
    v_kd = res.tile([P, S], BF, name="vkd", tag="vkd")  # [keys, dh] chunks

    # ---- preamble DMAs ----
    # Two constraints: (a) only 8 HWDGE semaphores exist, so more
    # in-flight DMAs than that forces sem-reuse ordering waits that stall
    # the issue queues; (b) the DMA engines round-robin across queues, so
    # bulk prefetch on a parallel queue starves the urgent proj0 feeds.
    # Everything bulk goes on ONE queue (sync) in priority order; only the
    # small tables ride a second queue.
    for ka, kb in ((0, 1), (1, 2), (2, 5), (5, 9)):
        nc.sync.dma_start(wqkv_sb[:, ka * 768:kb * 768], wqkv[:, ka:kb, :])
        nc.sync.dma_start(xt0_sb[:, ka * BLK:kb * BLK], xt[:, ka:kb, 0:BLK])
    nc.sync.dma_start(wqkv_sb[:, 9 * 768:16 * 768], wqkv[:, 9:16, :])
    nc.sync.dma_start(xt0_sb[:, 9 * BLK:16 * BLK], xt[:, 9:16, 0:BLK])
    # tables after the proj0 feeds (consumed by the first rope ~+30us)
    nc.sync.dma_start(cossin_t[:], cossin[:])
    nc.sync.dma_start(gqk_t[:], gqk[:])
    nc.sync.dma_start(xtr_sb[:, 0:16 * BLK], xt[:, :, BLK:2 * BLK])
    nc.sync.dma_start(
        xtr_sb[:, 16 * BLK:2 * 16 * BLK], xt[:, :, 2 * BLK:3 * BLK])
    nc.sync.dma_start(wo_sb[:], wo[:])
    nc.sync.dma_start(
        xtr_sb[:, 2 * 16 * BLK:3 * 16 * BLK], xt[:, :, 3 * BLK:4 * BLK])

    def xtile(nb, k):
        if nb == 0:
            return xt0_sb[:, k * BLK:(k + 1) * BLK]
        return xtr_sb[:, ((nb - 1) * 16 + k) * BLK:((nb - 1) * 16 + k + 1) * BLK]

    def wtile(k, m):
        return wqkv_sb[:, k * 768 + m * P:k * 768 + (m + 1) * P]

    # ---- per-block epilogue queues (rmsnorm + rope), pumped into gaps ----
    EPI = [deque() for _ in range(NBLK)]

    def pump(n=1):
        for _ in range(n):
            for nb in range(NBLK):
                if EPI[nb]:
                    nxt = EPI[nb].popleft()()
                    if nxt is not None:
                        EPI[nb].append(nxt)
                    break
            else:
                return

    def drain(nb):
        while EPI[nb]:
            nxt = EPI[nb].popleft()()
            if nxt is not None:
                EPI[nb].append(nxt)

    def rope_tile(dst, cols, rsf):
        """dst = (dst*cos + rot(dst)*sin) * rsf, in place; dst is the
        [P, BLK] column view; sins has the rotation sign baked into its
        first 64 rows."""
        t1 = rope_pool.tile([P, BLK], BF, name="t1", tag="t1")
        t2 = rope_pool.tile([P, BLK], BF, name="t2", tag="t2")
        nc.vector.tensor_copy(t2[0:64, :], dst[64:128, :])
        nc.vector.tensor_copy(t2[64:128, :], dst[0:64, :])
        nc.vector.tensor_tensor(t2[:], t2[:], sins_t[:, cols], MULT)
        nc.vector.tensor_tensor(t1[:], dst[:], cos_t[:, cols], MULT)
        nc.vector.tensor_tensor(t1[:], t1[:], t2[:], ADD)
        nc.vector.tensor_tensor(dst[:], t1[:], rsf[:], MULT)

    def stageA(nb, m, ps):
        cols = slice(nb * BLK, (nb + 1) * BLK)
        if m == 5:  # v: evict now, transpose to [keys, dh] chunks later
            nc.vector.tensor_copy(vT[:, cols], ps[:])

            def stageB_v():
                pst = pp.tile([P, BLK], BF, name="pst", tag="pp")
                for i in range(4):
                    c = nb * 4 + i
                    nc.tensor.transpose(pst[:, i * P:(i + 1) * P],
                                        vT[:, c * P:(c + 1) * P], ident[:])
                nc.scalar.copy(v_kd[:, cols], pst[:])
                return None

            EPI[nb].append(stageB_v)
            return
        if m < 4:
            dst, gsl, eps_t, escale = qT[m], gqk_t[:, 0:1], epsq_t, 1.0
        else:
            dst, gsl, eps_t, escale = kT, gqk_t[:, 1:2], epsk_t, 1.0 / P
        nc.scalar.activation(dst[:, cols], ps[:], Copy, bias=0.0, scale=gsl)
        sq = sq_pool.tile([P, BLK], BF, name="sq", tag="sq")
        nc.vector.tensor_tensor(sq[:], dst[:, cols], dst[:, cols], MULT)

        def stageB():
            # sumsq replicated across partitions by the ones-matmul.
            # rsqrt = exp(-0.5*ln(v)): ln and exp share ONE activation
            # table set (natural_log_exp_and_others) with the attention
            # exp, so the scalar engine never reloads tables (1.28us per
            # reload, dozens of sqrt<->exp switches otherwise). Both run
            # on the replicated [128,512] (free-size bound, same cost as
            # one row) so no row-extract/broadcast is ever needed.
            psr = pp.tile([P, BLK], F32, name="psr", tag="pp")
            nc.tensor.matmul(psr[:], ones_sq[:], sq[:], start=True, stop=True,
                             skip_group_check=True)
            lg = srt_pool.tile([P, BLK], F32, name="lg", tag="srt")
            nc.scalar.activation(lg[:], psr[:], Ln, bias=eps_t[:], scale=escale)
            rsf = rs_pool.tile([P, BLK], BF, name="rsf", tag="rsf")
            nc.scalar.activation(rsf[:], lg[:], Exp, bias=0.0, scale=-0.5)

            def stageC():
                cc = slice(nb * BLK, (nb + 1) * BLK)
                rope_tile(dst[:, cc], cc, rsf)
                return None

            return stageC

        EPI[nb].append(stageB)

    def proj_block(nb):
        if nb == 0:
            # k-outer: DMA-paced warmup; uses 6 psum banks across pools
            wide = scw.tile([P, 2 * BLK], F32, name="ps", tag="scw")
            psms = [wide[:, 0:BLK], wide[:, BLK:2 * BLK]]
            psms.append(attps.tile([P, BLK], F32, name="ps", tag="attps"))
            psms.append(sumps.tile([P, BLK], F32, name="ps", tag="sumps"))
            psms.append(pp.tile([P, BLK], F32, name="ps", tag="pp"))
            psms.append(pp.tile([P, BLK], F32, name="ps", tag="pp"))
            for k in range(16):
                for m in range(6):
                    nc.tensor.matmul(
                        psms[m], wtile(k, m), xtile(0, k),
                        start=(k == 0), stop=(k == 15), skip_group_check=True,
                    )
            for m in (5, 0, 1, 2, 3, 4):  # v first: frees its pp slot early
                stageA(0, m, psms[m])
        else:
            # k-tile first: its rope unlocks attention for all 4 heads.
            # The first two m-tiles borrow the free scw buffer (2 banks)
            # so they never wait on the previous segment's pp evictions.
            wide = scw.tile([P, 2 * BLK], F32, name="psw", tag="scw")
            for mi, m in enumerate((4, 0, 1, 2, 3, 5)):
                if mi < 2:
                    ps = wide[:, mi * BLK:(mi + 1) * BLK]
                else:
                    ps = pp.tile([P, BLK], F32, name="ps", tag="pp")[:]
                for k in range(16):
                    nc.tensor.matmul(
                        ps, wtile(k, m), xtile(nb, k),
                        start=(k == 0), stop=(k == 15), skip_group_check=True,
                    )
                stageA(nb, m, ps)
                pump(2)

    # ---- attention (software-pipelined) + Wo per query block ----
    def attn_head(h, qt, atts):
        """Emit scores/exp/AV for (h, qt). Score chunks are PAIRED into a
        [128,1024] 2-bank psum tile with ONE exp per pair."""
        nkc = 4 * qt + 4
        npair = nkc // 2
        q0 = qt * BLK
        ab = {}

        def pair(p):
            ps = scw.tile([P, 2 * BLK], F32, name="psS", tag="scw")
            exs = exp_pool.tile([P, 2 * BLK], BF, name="ex", tag="ex")
            offs = []
            for j in range(2):
                kc = 2 * p + j
                off = max(0, P * kc - q0)
                offs.append(off)
                nc.tensor.matmul(
                    ps[:, j * BLK + off:(j + 1) * BLK],
                    kT[:, kc * P:(kc + 1) * P], qT[h][:, q0 + off:q0 + BLK],
                    start=True, stop=True, skip_group_check=True,
                )
            nc.scalar.activation(exs[:, offs[0]:], ps[:, offs[0]:], Exp)
            # causal mask: zero the exp'd upper triangle of each diagonal
            # 128x128 corner on the vector engine — cheaper than the old
            # -30000 identity-matmul add inside the scores psum chain, and
            # the AV/rowsum matmuls only read exs a pair later (LAG)
            for j in range(2):
                kc = 2 * p + j
                if kc >= 4 * qt:
                    off = offs[j]
                    corner = exs[:, j * BLK + off:j * BLK + off + P]
                    nc.vector.tensor_tensor(corner, corner, mask_t[:], MULT)
            return p, offs, exs

        def av(p, offs, exs):
            if p == 0:
                ab["att"] = attps.tile([P, BLK], F32, name="psA", tag="attps")
                ab["sum"] = sumps.tile([P, BLK], F32, name="psB", tag="sumps")
            for j in range(2):
                kc = 2 * p + j
                off = offs[j]
                exv = exs[:, j * BLK + off:(j + 1) * BLK]
                nc.tensor.matmul(
                    ab["att"][:, off:], v_kd[:, kc * P:(kc + 1) * P], exv,
                    start=(kc == 0), stop=(kc == nkc - 1), skip_group_check=True,
                )
                nc.tensor.matmul(
                    ab["sum"][:, off:], ones_sq[:], exv,
                    start=(kc == 0), stop=(kc == nkc - 1), skip_group_check=True,
                )

        # NO pumping inside attention: the scalar engine is ~95% busy with
        # the exp stream here (1.1us exp vs 1.28us PE per pair) and the
        # DVE must run the norm promptly to free the att/sum psum banks —
        # epilogue work injected into either queue stalls the PE.
        pend = []
        for p in range(npair):
            pend.append(pair(p))
            if len(pend) > 1:
                av(*pend.pop(0))
        while pend:
            av(*pend.pop(0))

        # normalize: rowsum psum is replicated across partitions, so one
        # reciprocal + one multiply straight out of the att psum.
        rrep = nrm_pool.tile([P, BLK], F32, name="rrep", tag="rrep")
        nc.vector.reciprocal_approx_fast(rrep[:], ab["sum"][:])
        a = att_pool.tile([P, BLK], BF, name=f"att{h}", tag=f"att{h}")
        nc.vector.tensor_tensor(a[:], ab["att"][:], rrep[:], MULT)
        atts[h] = a

    def wo_tc4(qt, tc4, atts, tail=False):
        """One 128-query group of the Wo projection for query block qt.
        Interleaved between attention heads of block qt+1: the ~3.4us of
        exp-free PE work absorbs the previous head's norm latency and
        gives the scalar engine slack for pumped epilogue work."""
        q0 = qt * BLK
        osb = osb_pool.tile([P, D], BF, name="osb", tag="osb")
        for et in range(4):
            ps = pp.tile([P, 512], F32, name="pso", tag="pp")
            for h2 in range(HPC):
                nc.tensor.matmul(
                    ps[:], atts[h2][:, tc4 * P:(tc4 + 1) * P],
                    wo_sb[:, h2 * D + et * 512:h2 * D + (et + 1) * 512],
                    start=(h2 == 0), stop=(h2 == HPC - 1), skip_group_check=True,
                )
            # evicts on vector only: scalar must stay clear for the exp
            # stream (gpsimd can't read PSUM)
            nc.vector.tensor_copy(osb[:, et * 512:(et + 1) * 512], ps[:])
            if tail:
                nc.sync.dma_start(
                    out[q0 + tc4 * P:q0 + (tc4 + 1) * P, et * 512:(et + 1) * 512],
                    osb[:, et * 512:(et + 1) * 512])
        if not tail:
            nc.sync.dma_start(out[q0 + tc4 * P:q0 + (tc4 + 1) * P, :], osb[:])
        pump(2)

    def attn_block(qt, prev_atts):
        atts = [None] * HPC
        for h in range(HPC):
            attn_head(h, qt, atts)
            if prev_atts is not None:
                wo_tc4(qt - 1, h, prev_atts)
        return atts

    # ---- interleaved schedule ----
    proj_block(0)
    proj_block(1)
    drain(0)
    atts0 = attn_block(0, None)
    proj_block(2)
    drain(1)
    atts1 = attn_block(1, atts0)
    proj_block(3)
    drain(2)
    atts2 = attn_block(2, atts1)
    drain(3)
    atts3 = attn_block(3, atts2)
    for tc4 in range(4):
        wo_tc4(3, tc4, atts3, tail=(tc4 == 3))


_NC_CACHE = None


def _single_act_table(nc):
    """Make every activation resolve to the one table set that holds exp,
    ln AND copy (natural_log_exp_and_others). The stock assignment maps
    each function to the FIRST containing set (exp->0, ln->5), emitting an
    alternating 1.28us ACT_TABLE_LOAD per rsqrt<->softmax switch — dozens
    per kernel. Emptying the other sets (indices preserved, so the BIR
    set-id still matches act_info.json) collapses it to one load."""
    import types
    from concourse.hw_specs import get_activation_tables

    orig = get_activation_tables(nc.m.arch)
    keep = "natural_log_exp_and_others"
    assert keep in orig, sorted(orig)
    filtered = {n: (fns if n == keep else set()) for n, fns in orig.items()}

    def patched(self):
        has_activation = any(
            isinstance(i, mybir.InstActivation)
            for b in self.main_func.blocks
            for i in b.instructions
        )
        if not has_activation:
            return
        import bass_rust as _bass_rust
        _bass_rust.insert_act_table_loads(self, list(filtered.items()))

    nc.insert_act_table_loads = types.MethodType(patched, nc)


def build_nc():
    global _NC_CACHE
    if _NC_CACHE is not None:
        return _NC_CACHE
    nc = bacc.Bacc(None, target_bir_lowering=False)
    _single_act_table(nc)
    xt = nc.dram_tensor("xt", [P, 16, S], BF, kind="ExternalInput")
    wqkv = nc.dram_tensor("wqkv", [P, 16, 768], BF, kind="ExternalInput")
    wo = nc.dram_tensor("wo", [P, HPC * D], BF, kind="ExternalInput")
    cossin = nc.dram_tensor("cossin", [P, 2 * S + P], BF, kind="ExternalInput")
    gqk = nc.dram_tensor("gqk", [P, 2], F32, kind="ExternalInput")
    out = nc.dram_tensor("out", [S, D], BF, kind="ExternalOutput")
    with tile.TileContext(nc) as tc:
        with ExitStack() as ctx:
            _body(ctx, tc, xt[:], wqkv[:], wo[:], cossin[:], gqk[:], out[:])
    nc.compile()
    _NC_CACHE = nc
    return nc


def _host_tables():
    pos = np.arange(S, dtype=np.float64)
    inv_freq = 1.0 / (ROPE_THETA ** (np.arange(0, DH, 2, dtype=np.float64) / DH))
    ang = pos[:, None] * inv_freq[None, :]  # [S, 64]
    cos_s = np.concatenate([np.cos(ang), np.cos(ang)], axis=-1)  # [S, 128]
    sin_s = np.concatenate([np.sin(ang), np.sin(ang)], axis=-1)
    cos_full = np.ascontiguousarray(cos_s.T)  # [128, S]
    sins = sin_s.T.copy()
    sins[0:64] *= -1.0  # rotation sign baked in
    j = np.arange(P)[:, None]
    i = np.arange(P)[None, :]
    masktri = np.where(j <= i, 1.0, 0.0)  # [keys, queries] causal 0/1
    # one [128, 2S+128] blob: [cos | sins | mask] — a single preamble DMA
    cossin = np.concatenate([cos_full, sins, masktri], axis=1).astype(BFNP)
    return cossin


def kernel(qkv, Wq, Wk, Wv, Wo, q_gamma, k_gamma):
    qkv = np.asarray(qkv, dtype=np.float32)
    Wq = np.asarray(Wq, dtype=np.float32)
    Wk = np.asarray(Wk, dtype=np.float32)
    Wv = np.asarray(Wv, dtype=np.float32)
    Wo = np.asarray(Wo, dtype=np.float32)
    q_gamma = np.asarray(q_gamma, dtype=np.float32)
    k_gamma = np.asarray(k_gamma, dtype=np.float32)

    nc = build_nc()
    cossin = _host_tables()
    gqk = np.ascontiguousarray(
        np.stack([q_gamma, k_gamma], axis=1)).astype(np.float32)  # [128, 2]
    # x^T tiles in [p, k, s] layout: element [p, k, s] = qkv[b].T[128k+p, s]
    xts = [
        np.ascontiguousarray(
            qkv[b].T.reshape(16, P, S).transpose(1, 0, 2)
        ).astype(BFNP)
        for b in range(B)
    ]

    in_maps = []
    for c in range(NCORES):
        b, g = c // 4, c % 4
        wq_c = Wq[4 * g * DH:(4 * g + 4) * DH, :]  # [512, D]
        wk_c = Wk[g * DH:(g + 1) * DH, :]  # [128, D]
        wv_c = Wv[g * DH:(g + 1) * DH, :]
        wqkv_c = np.concatenate([wq_c, wk_c, wv_c], axis=0).T  # [D, 768]
        wqkv_c = np.ascontiguousarray(
            wqkv_c.reshape(16, P, 768).transpose(1, 0, 2)).astype(BFNP)  # [128,16,768]
        wo_c = np.stack(
            [np.ascontiguousarray(Wo[:, (4 * g + h) * DH:(4 * g + h + 1) * DH].T)
             for h in range(HPC)]
        )  # [4, 128, D]
        wo_c = np.ascontiguousarray(
            wo_c.transpose(1, 0, 2).reshape(P, HPC * D)).astype(BFNP)
        in_maps.append({
            "xt": xts[b], "wqkv": wqkv_c, "wo": wo_c,
            "cossin": cossin, "gqk": gqk,
        })

    res = run_bass_kernel_spmd(nc, in_maps, core_ids=list(range(NCORES)))
    full = np.empty((B, S, D), np.float32)
    for b in range(B):
        acc = res.results[4 * b]["out"].astype(np.float32)
        for g in range(1, 4):
            acc += res.results[4 * b + g]["out"].astype(np.float32)
        full[b] = acc
    return full
